# revision 28
# baseline (speedup 1.0000x reference)
"""DIN-style attention (MLP over [qt, k, qt-k, qt*k] + masked softmax) on 8 TRN2 cores.

Data-parallel over batch: each core handles 512 of 4096 rows.

Structure (v3, ACT-bound design with transposed layer 2):
  - sigmoid -> tanh identity: sigma(x) = 0.5 + 0.5*tanh(x/2). The 0.5 factors fold
    into W2, b2, Wf on the host; the constant logit shift cancels in softmax.
    Tanh and Exp share one activation table set -> no table swaps, ever.
  - W1 is algebraically combined on-device: info@W1 = qp@(W1q+W1m) + k@(W1k-W1m)
    + (qp*k)@W1p, so the (qt-k) features never materialize.
  - L1: per group of 4 t's, 12 K=32 matmuls at 4 distinct PE row strips (run
    concurrently on HW) into a 4-bank PSUM tile [80, 4, 512]; sigma1 is ONE tanh
    per 4 t's (free dim 2048, 80 lanes).
  - L2 is TRANSPOSED: stationary = bf16 a1 chunk [81, 128] (row 80 = ones, which
    delivers the bias via W2aug's last row), moving = W2aug [81, 40]. Output is
    [128 b-partitions, 40 h2-free] -> sigma2 runs at FULL 128 lanes: one tanh per
    4 t's at free dim 640 (4x cheaper than feature-major). bf16 stationary
    triggers Fast Weight Load (128 cols).
  - L3 matmuls are GONE: logit[b,t] = sum_h wf_h * a2[b,t,h] is a DVE
    multiply + segmented reduce straight into the [128 b, t] softmax layout
    (SBUF, no PSUM needed).
  - Emission is software-pipelined so ACT (the bottleneck engine) never stalls.
"""

import contextlib

import numpy as np

import concourse.bacc as bacc
import concourse.mybir as mybir
import concourse.tile as tile
from concourse.bass_utils import run_bass_kernel_spmd

N_CORES = 8
B, T, D = 4096, 200, 32
BC = B // N_CORES  # 512 rows per core
H1, H2 = 80, 40
NBLK = BC // 128   # 4 blocks of 128 b's
NEG_BIG = float(np.float32(-2.0**32 + 1.0))

S2_PADDED = True   # sigma2 reads the full padded [.,1024] span vs strided 640
K_BF16 = True      # carry k / qp / qk / W1 through the L1 matmuls in bf16

F32 = mybir.dt.float32
F32R = mybir.dt.float32r
BF16 = mybir.dt.bfloat16
I8 = mybir.dt.int8
AF = mybir.ActivationFunctionType
ALU = mybir.AluOpType
AX = mybir.AxisListType


def _emit(nc, tc, es, d, TT, repeat=1):
    NG = TT // 4
    const = es.enter_context(tc.tile_pool(name="const", bufs=1))
    ktp = es.enter_context(tc.tile_pool(name="ktp", bufs=4))
    qkp = es.enter_context(tc.tile_pool(name="qkp", bufs=3))
    a2p = es.enter_context(tc.tile_pool(name="a2p", bufs=2))
    prp = es.enter_context(tc.tile_pool(name="prp", bufs=2))
    ps1p = es.enter_context(tc.tile_pool(name="ps1p", bufs=1, space="PSUM"))
    ps2p = es.enter_context(tc.tile_pool(name="ps2p", bufs=1, space="PSUM"))

    # ---- static tiles ----
    w1raw = const.tile([4 * D, H1], F32R)
    wrep = const.tile([128, 3 * H1], F32R)  # strip j: [W1q+W1m | W1k-W1m | W1p]
    wq = const.tile([D, D], F32R)
    w2raw = const.tile([H1 + 1, H2], F32)
    w2aug = const.tile([H1 + 1, H2], BF16)  # [0.5*W2 ; c2] (bias via ones-row)
    wfraw = const.tile([128, H2], F32)
    wfb = const.tile([128, H2], BF16)       # 0.5*Wf replicated on all partitions
    b1h = const.tile([H1, 1], F32)          # b1 / 2
    bqs = const.tile([D, 1], F32)
    als = const.tile([D, 1], F32)
    qts = const.tile([D, BC], F32R)
    qp4 = const.tile([128, BC], F32R)       # qp^T replicated at 4 strips
    mki = const.tile([128, NBLK, TT], I8)
    negb = const.tile([128, NBLK, TT], F32)
    tmpr = const.tile([D, BC], F32)
    tmpa = const.tile([D, BC], F32)
    tmpb = const.tile([D, BC], F32)
    # a1 double buffer: [81, 4, BC] bf16, row 80 = ones (bias row)
    a1A = const.tile([H1 + 1, 4, BC], BF16)
    a1B = const.tile([H1 + 1, 4, BC], BF16)
    logt = const.tile([128, NBLK, TT], F32)
    mx = const.tile([128, NBLK], F32)
    sums = const.tile([128, NBLK], F32)
    rin = const.tile([128, NBLK], F32)
    expv = const.tile([128, NBLK, TT], F32)
    att = const.tile([128, NBLK, TT], F32)

    nc.sync.dma_start(out=w1raw, in_=d["W1"])
    nc.sync.dma_start(out=wq, in_=d["Wq"])
    nc.sync.dma_start(out=w2raw, in_=d["W2aug"])
    nc.sync.dma_start(out=wfraw, in_=d["wfb"])
    nc.vector.tensor_copy(w2aug, w2raw)
    nc.vector.tensor_copy(wfb, wfraw)
    nc.sync.dma_start(out=b1h, in_=d["b1h"])
    nc.sync.dma_start(out=bqs, in_=d["bq"])
    nc.sync.dma_start(out=als, in_=d["alpha"])
    nc.sync.dma_start(out=qts, in_=d["qT"])
    nc.sync.dma_start(out=mki, in_=d["mki"])
    # ones bias-row at partition 80: DVE can't start mid-strip -> memset at
    # partition 0 and DMA the row into place
    ones1 = const.tile([1, 4 * BC], BF16)
    nc.vector.memset(ones1, 1.0)
    nc.sync.dma_start(
        out=a1A[H1:H1 + 1, :, :], in_=ones1.rearrange("p (j b) -> p j b", j=4))
    nc.sync.dma_start(
        out=a1B[H1:H1 + 1, :, :], in_=ones1.rearrange("p (j b) -> p j b", j=4))

    # dummy tanh: hoists the activation-table load (exp_and_others covers both
    # Tanh and Exp) into setup so the first real tanh doesn't serialize on it
    nc.scalar.activation(tmpr[:, 0:1], bqs, AF.Tanh)

    # combined W1 blocks, then replicate to strips 1..3.
    # HW verifier (NCC_IBIR297) requires equal base partitions for 2-input DVE
    # ops, so align the blocks to base 0 first; cross-partition moves go via DMA.
    t32 = const.tile([32, H1], F32R)
    t64 = const.tile([32, H1], F32R)
    nc.sync.dma_start(out=t32, in_=d["W1"][32:64, :])
    nc.sync.dma_start(out=t64, in_=d["W1"][64:96, :])
    nc.vector.tensor_add(wrep[0:32, 0:H1], w1raw[0:32, :], t64)
    nc.vector.tensor_sub(wrep[0:32, H1:2 * H1], t32, t64)
    nc.sync.dma_start(out=wrep[0:32, 2 * H1:3 * H1], in_=d["W1"][96:128, :])
    for j in range(1, 4):
        nc.sync.dma_start(out=wrep[32 * j:32 * j + 32, :], in_=wrep[0:32, :])

    # qp^T = prelu(Wq^T @ q^T + bq, alpha)
    ps0 = ps1p.tile([D, BC], F32, tag="ps1")
    nc.tensor.matmul(ps0, wq, qts, start=True, stop=True)
    nc.vector.tensor_scalar(tmpr, ps0, bqs, 0.0, op0=ALU.add, op1=ALU.max)
    nc.vector.tensor_scalar(tmpa, ps0, bqs, 0.0, op0=ALU.add, op1=ALU.min)
    nc.vector.tensor_scalar(tmpb, tmpa, als, None, op0=ALU.mult)
    nc.vector.tensor_add(qp4[0:32, :], tmpr, tmpb)
    for j in range(1, 4):
        nc.sync.dma_start(out=qp4[32 * j:32 * j + 32, :], in_=qp4[0:32, :])

    nc.vector.memset(negb, NEG_BIG)

    if K_BF16:
        wrep_b = const.tile([128, 3 * H1], BF16)
        qp4_b = const.tile([128, BC], BF16)
        nc.vector.tensor_copy(wrep_b, wrep.bitcast(F32))
        nc.vector.tensor_copy(qp4_b, qp4.bitcast(F32))
    else:
        wrep_b, qp4_b = wrep, qp4

    # persistent PSUM tiles. ps2 slots are padded to 64 so no matmul output
    # crosses a PSUM bank boundary; layout [128, j(t), c(blk), 64].
    ps1 = ps1p.tile([H1, 4, BC], F32, tag="ps1")       # 4 banks
    ps2A = ps2p.tile([128, 4, NBLK, 64], F32)          # 2 banks
    ps2B = ps2p.tile([128, 4, NBLK, 64], F32)          # 2 banks
    # pad columns are never matmul-written; zero once so tanh(junk) can't NaN
    nc.vector.memset(ps2A, 0.0)
    nc.vector.memset(ps2B, 0.0)

    for _rep in range(repeat):
        _main_pass(nc, d, TT, NG, ktp, qkp, a2p, prp, ps1, (ps2A, ps2B),
                   (a1A, a1B), logt, wrep_b, w2aug, wfb, b1h, qp4_b, mki, negb,
                   mx, sums, rin, expv, att)


def _main_pass(nc, d, TT, NG, ktp, qkp, a2p, prp, ps1, ps2s, a1s_ab, logt,
               wrep, w2aug, wfb, b1h, qp4, mki, negb, mx, sums, rin, expv, att):
    kts, qks, a2s = {}, {}, {}

    KD = BF16 if K_BF16 else F32R

    def dma_kt(g):
        kt = ktp.tile([128, BC], KD)
        nc.sync.dma_start(
            out=kt, in_=d["kT"][4 * g:4 * g + 4].rearrange("tj f b -> (tj f) b"))
        kts[g] = kt

    def emit_qk(g):
        qk = qkp.tile([128, BC], KD)
        nc.vector.tensor_mul(qk, qp4, kts[g])
        qks[g] = qk

    def emit_l1(g):
        kt, qk = kts.pop(g), qks.pop(g)
        for j in range(4):
            s = slice(32 * j, 32 * j + 32)
            tp = (32 * j, 0)
            p1 = ps1[:, j, :]
            nc.tensor.matmul(p1, wrep[s, 0:H1], qp4[s, :], start=True,
                             stop=False, tile_position=tp)
            nc.tensor.matmul(p1, wrep[s, H1:2 * H1], kt[s, :], start=False,
                             stop=False, tile_position=tp)
            nc.tensor.matmul(p1, wrep[s, 2 * H1:3 * H1], qk[s, :], start=False,
                             stop=True, tile_position=tp)

    def emit_s1(g):
        a1 = a1s_ab[g % 2]
        nc.scalar.activation(a1[0:H1, :, :], ps1, AF.Tanh, bias=b1h, scale=0.5)

    def emit_l2(g):
        a1 = a1s_ab[g % 2]
        ps2 = ps2s[g % 2]
        for j in range(4):
            for c in range(NBLK):
                nc.tensor.matmul(
                    ps2[:, j, c, 0:H2],
                    a1[:, j, 128 * c:128 * c + 128],
                    w2aug,
                    start=True,
                    stop=True,
                )

    def emit_s2(g):
        ps2 = ps2s[g % 2]
        a2 = a2p.tile([128, 4, NBLK, 64], BF16)
        if S2_PADDED:
            nc.scalar.activation(a2, ps2, AF.Tanh, scale=0.5)
        else:
            nc.scalar.activation(a2[:, :, :, 0:H2], ps2[:, :, :, 0:H2], AF.Tanh,
                                 scale=0.5)
        a2s[g] = a2

    def emit_logits(g):
        a2 = a2s.pop(g)
        pr = prp.tile([128, 4, NBLK, 64], BF16)
        nc.vector.tensor_mul(
            pr[:, :, :, 0:H2], a2[:, :, :, 0:H2],
            wfb.unsqueeze(1).unsqueeze(1).broadcast_to([128, 4, NBLK, H2]))
        # out view [128, j, c] of logt[:, c, t0+j] via free-dim transpose
        nc.vector.tensor_reduce(
            logt[:, :, 4 * g:4 * g + 4].transpose([0, 2, 1]),
            pr[:, :, :, 0:H2], axis=AX.X, op=ALU.add)

    # ---- software-pipelined main loop ----
    dma_kt(0)
    emit_qk(0)
    emit_l1(0)
    emit_s1(0)
    for g in range(NG):
        if g + 1 < NG:
            dma_kt(g + 1)
            emit_qk(g + 1)
            emit_l1(g + 1)
        emit_l2(g)
        if g >= 1:
            emit_logits(g - 1)
        if g + 1 < NG:
            emit_s1(g + 1)
        emit_s2(g)
    emit_logits(NG - 1)

    # ---- masked softmax over t ----
    # Per-block pipeline: exp (ACT) || sum+recip+scale (DVE) || out-DMA of the
    # previous block. No ACT accum_out (its read-accumulator aux op is ~0.7us);
    # the sum runs on the otherwise-idle DVE instead.
    outv = d["out"].rearrange("(blk p) t -> p blk t", blk=NBLK)
    nc.vector.copy_predicated(logt, mki, negb)
    nc.vector.tensor_reduce(mx, logt, axis=AX.X, op=ALU.max, negate=True)
    for blk in range(NBLK):
        nc.scalar.activation(
            expv[:, blk, :],
            logt[:, blk, :],
            AF.Exp,
            bias=mx[:, blk:blk + 1],
        )
        nc.vector.tensor_reduce(
            sums[:, blk:blk + 1], expv[:, blk, :], axis=AX.X, op=ALU.add)
        nc.vector.reciprocal(rin[:, blk:blk + 1], sums[:, blk:blk + 1])
        nc.vector.tensor_scalar(
            att[:, blk, :], expv[:, blk, :], rin[:, blk:blk + 1], None,
            op0=ALU.mult)
        nc.sync.dma_start(out=outv[:, blk, :], in_=att[:, blk, :])


def build(TT=T, repeat=1):
    nc = bacc.Bacc("TRN2", target_bir_lowering=False, debug=False,
                   num_devices=N_CORES)
    d = {
        "kT": nc.dram_tensor("kT", [TT, D, BC], BF16 if K_BF16 else F32R,
                             kind="ExternalInput").ap(),
        "qT": nc.dram_tensor("qT", [D, BC], F32R, kind="ExternalInput").ap(),
        "mki": nc.dram_tensor("mki", [128, NBLK, TT], I8, kind="ExternalInput").ap(),
        "Wq": nc.dram_tensor("Wq", [D, D], F32R, kind="ExternalInput").ap(),
        "bq": nc.dram_tensor("bq", [D, 1], F32, kind="ExternalInput").ap(),
        "alpha": nc.dram_tensor("alpha", [D, 1], F32, kind="ExternalInput").ap(),
        "W1": nc.dram_tensor("W1", [4 * D, H1], F32R, kind="ExternalInput").ap(),
        "b1h": nc.dram_tensor("b1h", [H1, 1], F32, kind="ExternalInput").ap(),
        "W2aug": nc.dram_tensor("W2aug", [H1 + 1, H2], F32,
                                kind="ExternalInput").ap(),
        "wfb": nc.dram_tensor("wfb", [128, H2], F32, kind="ExternalInput").ap(),
        "out": nc.dram_tensor("out", [BC, TT], F32, kind="ExternalOutput").ap(),
    }
    with tile.TileContext(nc) as tc:
        with contextlib.ExitStack() as es:
            _emit(nc, tc, es, d, TT, repeat=repeat)
    nc.compile()
    return nc


def prepare(q, k, mask, Wq, bq, alpha, W1, b1, W2, b2, Wf, bf=None):
    """Varlen packing: per batch row keep only its unmasked t's (plus padding to
    the global max count, rounded to a multiple of 4). Pure index manipulation.
    Returns (in_maps, TT, tidx)."""
    mask_np = np.asarray(mask)
    cnt = (mask_np != 0).sum(1)                      # unmasked count per row
    if cnt.min() == 0:
        # a fully-masked row needs the uniform-softmax semantics; identity
        # "compaction" reproduces the uncompacted kernel exactly
        TT = T
        tidx = np.ascontiguousarray(np.tile(np.arange(T), (B, 1)))
        pad = mask_np == 0
    else:
        TT = int(-(-int(cnt.max()) // 4) * 4)        # round up to x4
        TT = max(TT, 8)
        order = np.argsort(mask_np == 0, axis=1, kind="stable")  # unmasked first
        tidx = np.ascontiguousarray(order[:, :TT])   # [B, TT]
        pad = (np.arange(TT)[None, :] >= cnt[:, None])  # True on pad slots
    kc = np.take_along_axis(np.asarray(k), tidx[:, :, None], axis=1)  # [B, TT, D]

    W2 = np.asarray(W2, np.float32)
    b2 = np.asarray(b2, np.float32)
    Wf = np.asarray(Wf, np.float32).reshape(H2)
    b1 = np.asarray(b1, np.float32)
    # sigma(x) = 0.5 + 0.5*tanh(x/2) folding:
    #   z2 = W2^T sig1 + b2 = (0.5 W2)^T t1 + (b2 + 0.5 * sum_h W2[h,:])
    #   logit = Wf . sig2 (+const) = (0.5 Wf) . t2 + const   (const drops in softmax)
    c2 = b2 + 0.5 * W2.sum(axis=0)                   # [H2]
    W2aug = np.concatenate([0.5 * W2, c2[None, :]], axis=0)  # [81, 40]
    wfb = np.tile((0.5 * Wf)[None, :], (128, 1))     # [128, 40]

    common = {
        "Wq": np.ascontiguousarray(Wq, np.float32),
        "bq": np.ascontiguousarray(bq, np.float32).reshape(D, 1),
        "alpha": np.ascontiguousarray(alpha, np.float32).reshape(D, 1),
        "W1": np.ascontiguousarray(W1, np.float32),
        "b1h": np.ascontiguousarray(0.5 * b1, np.float32).reshape(H1, 1),
        "W2aug": np.ascontiguousarray(W2aug, np.float32),
        "wfb": np.ascontiguousarray(wfb, np.float32),
    }
    in_maps = []
    for c in range(N_CORES):
        sl = slice(c * BC, (c + 1) * BC)
        if K_BF16:
            import ml_dtypes
            kcc = np.ascontiguousarray(
                kc[sl].transpose(1, 2, 0).astype(ml_dtypes.bfloat16))
        else:
            kcc = np.ascontiguousarray(kc[sl].transpose(1, 2, 0), np.float32)
        qc = np.ascontiguousarray(np.asarray(q)[sl].T, np.float32)  # [D, BC]
        mc = pad[sl].astype(np.int8)
        mc = np.ascontiguousarray(mc.reshape(NBLK, 128, TT).transpose(1, 0, 2))
        m = dict(common)
        m.update({"kT": kcc, "qT": qc, "mki": mc})
        in_maps.append(m)
    return in_maps, TT, tidx


def postprocess(results, TT, tidx):
    attc = np.empty((B, TT), np.float32)
    for c in range(N_CORES):
        attc[c * BC:(c + 1) * BC] = results[c]["out"]
    out = np.zeros((B, T), np.float32)
    np.put_along_axis(out, tidx, attc, axis=1)
    return out.reshape(B, 1, T)


_NC_CACHE = {}


def kernel(**inputs):
    in_maps, TT, tidx = prepare(
        inputs["q"], inputs["k"], inputs["mask"], inputs["Wq"], inputs["bq"],
        inputs["alpha"], inputs["W1"], inputs["b1"], inputs["W2"], inputs["b2"],
        inputs["Wf"],
    )
    if TT not in _NC_CACHE:
        _NC_CACHE[TT] = build(TT=TT)
    nc = _NC_CACHE[TT]
    res = run_bass_kernel_spmd(nc, in_maps, core_ids=list(range(N_CORES)))
    return postprocess(res.results, TT, tidx)


# revision 30
# speedup vs baseline: 1.0405x; 1.0405x over previous
"""DIN-style attention (MLP over [qt, k, qt-k, qt*k] + masked softmax) on 8 TRN2 cores.

Data-parallel over batch: each core handles 512 of 4096 rows.

Structure (v3, ACT-bound design with transposed layer 2):
  - sigmoid -> tanh identity: sigma(x) = 0.5 + 0.5*tanh(x/2). The 0.5 factors fold
    into W2, b2, Wf on the host; the constant logit shift cancels in softmax.
    Tanh and Exp share one activation table set -> no table swaps, ever.
  - W1 is algebraically combined on-device: info@W1 = qp@(W1q+W1m) + k@(W1k-W1m)
    + (qp*k)@W1p, so the (qt-k) features never materialize.
  - L1: per group of 4 t's, 12 K=32 matmuls at 4 distinct PE row strips (run
    concurrently on HW) into a 4-bank PSUM tile [80, 4, 512]; sigma1 is ONE tanh
    per 4 t's (free dim 2048, 80 lanes).
  - L2 is TRANSPOSED: stationary = bf16 a1 chunk [81, 128] (row 80 = ones, which
    delivers the bias via W2aug's last row), moving = W2aug [81, 40]. Output is
    [128 b-partitions, 40 h2-free] -> sigma2 runs at FULL 128 lanes: one tanh per
    4 t's at free dim 640 (4x cheaper than feature-major). bf16 stationary
    triggers Fast Weight Load (128 cols).
  - L3 matmuls are GONE: logit[b,t] = sum_h wf_h * a2[b,t,h] is a DVE
    multiply + segmented reduce straight into the [128 b, t] softmax layout
    (SBUF, no PSUM needed).
  - Emission is software-pipelined so ACT (the bottleneck engine) never stalls.
"""

import contextlib

import numpy as np

import concourse.bacc as bacc
import concourse.mybir as mybir
import concourse.tile as tile
from concourse.bass_utils import run_bass_kernel_spmd

N_CORES = 8
B, T, D = 4096, 200, 32
BC = B // N_CORES  # 512 rows per core
H1, H2 = 80, 40
NBLK = BC // 128   # 4 blocks of 128 b's
NEG_BIG = float(np.float32(-2.0**32 + 1.0))

S2_PADDED = True   # sigma2 reads the full padded [.,1024] span vs strided 640
TAIL_PIPE = True   # per-block softmax tail (DVE sums) vs monolithic ACT accum
K_BF16 = True      # carry k / qp / qk / W1 through the L1 matmuls in bf16

F32 = mybir.dt.float32
F32R = mybir.dt.float32r
BF16 = mybir.dt.bfloat16
I8 = mybir.dt.int8
AF = mybir.ActivationFunctionType
ALU = mybir.AluOpType
AX = mybir.AxisListType


def _emit(nc, tc, es, d, TT, repeat=1):
    NG = TT // 4
    const = es.enter_context(tc.tile_pool(name="const", bufs=1))
    ktp = es.enter_context(tc.tile_pool(name="ktp", bufs=4))
    qkp = es.enter_context(tc.tile_pool(name="qkp", bufs=3))
    a2p = es.enter_context(tc.tile_pool(name="a2p", bufs=2))
    prp = es.enter_context(tc.tile_pool(name="prp", bufs=2))
    ps1p = es.enter_context(tc.tile_pool(name="ps1p", bufs=1, space="PSUM"))
    ps2p = es.enter_context(tc.tile_pool(name="ps2p", bufs=1, space="PSUM"))

    # ---- static tiles ----
    w1raw = const.tile([4 * D, H1], F32R)
    wrep = const.tile([128, 3 * H1], F32R)  # strip j: [W1q+W1m | W1k-W1m | W1p]
    wq = const.tile([D, D], F32R)
    w2raw = const.tile([H1 + 1, H2], F32)
    w2aug = const.tile([H1 + 1, H2], BF16)  # [0.5*W2 ; c2] (bias via ones-row)
    wfraw = const.tile([128, H2], F32)
    wfb = const.tile([128, H2], BF16)       # 0.5*Wf replicated on all partitions
    b1h = const.tile([H1, 1], F32)          # b1 / 2
    bqs = const.tile([D, 1], F32)
    als = const.tile([D, 1], F32)
    qts = const.tile([D, BC], F32R)
    qp4 = const.tile([128, BC], F32R)       # qp^T replicated at 4 strips
    mki = const.tile([128, NBLK, TT], I8)
    negb = const.tile([128, NBLK, TT], F32)
    tmpr = const.tile([D, BC], F32)
    tmpa = const.tile([D, BC], F32)
    tmpb = const.tile([D, BC], F32)
    # a1 double buffer: [81, 4, BC] bf16, row 80 = ones (bias row)
    a1A = const.tile([H1 + 1, 4, BC], BF16)
    a1B = const.tile([H1 + 1, 4, BC], BF16)
    logt = const.tile([128, NBLK, TT], F32)
    mx = const.tile([128, NBLK], F32)
    sums = const.tile([128, NBLK], F32)
    rin = const.tile([128, NBLK], F32)
    expv = const.tile([128, NBLK, TT], F32)
    att = const.tile([128, NBLK, TT], F32)

    nc.sync.dma_start(out=w1raw, in_=d["W1"])
    nc.sync.dma_start(out=wq, in_=d["Wq"])
    nc.sync.dma_start(out=w2raw, in_=d["W2aug"])
    nc.sync.dma_start(out=wfraw, in_=d["wfb"])
    nc.vector.tensor_copy(w2aug, w2raw)
    nc.vector.tensor_copy(wfb, wfraw)
    nc.sync.dma_start(out=b1h, in_=d["b1h"])
    nc.sync.dma_start(out=bqs, in_=d["bq"])
    nc.sync.dma_start(out=als, in_=d["alpha"])
    nc.sync.dma_start(out=qts, in_=d["qT"])
    nc.sync.dma_start(out=mki, in_=d["mki"])
    # ones bias-row at partition 80: DVE can't start mid-strip -> memset at
    # partition 0 and DMA the row into place
    ones1 = const.tile([1, 4 * BC], BF16)
    nc.vector.memset(ones1, 1.0)
    nc.sync.dma_start(
        out=a1A[H1:H1 + 1, :, :], in_=ones1.rearrange("p (j b) -> p j b", j=4))
    nc.sync.dma_start(
        out=a1B[H1:H1 + 1, :, :], in_=ones1.rearrange("p (j b) -> p j b", j=4))

    # dummy tanh: hoists the activation-table load (exp_and_others covers both
    # Tanh and Exp) into setup so the first real tanh doesn't serialize on it
    nc.scalar.activation(tmpr[:, 0:1], bqs, AF.Tanh)

    # combined W1 blocks, then replicate to strips 1..3.
    # HW verifier (NCC_IBIR297) requires equal base partitions for 2-input DVE
    # ops, so align the blocks to base 0 first; cross-partition moves go via DMA.
    t32 = const.tile([32, H1], F32R)
    t64 = const.tile([32, H1], F32R)
    nc.sync.dma_start(out=t32, in_=d["W1"][32:64, :])
    nc.sync.dma_start(out=t64, in_=d["W1"][64:96, :])
    nc.vector.tensor_add(wrep[0:32, 0:H1], w1raw[0:32, :], t64)
    nc.vector.tensor_sub(wrep[0:32, H1:2 * H1], t32, t64)
    nc.sync.dma_start(out=wrep[0:32, 2 * H1:3 * H1], in_=d["W1"][96:128, :])
    for j in range(1, 4):
        nc.sync.dma_start(out=wrep[32 * j:32 * j + 32, :], in_=wrep[0:32, :])

    # qp^T = prelu(Wq^T @ q^T + bq, alpha)
    ps0 = ps1p.tile([D, BC], F32, tag="ps1")
    nc.tensor.matmul(ps0, wq, qts, start=True, stop=True)
    nc.vector.tensor_scalar(tmpr, ps0, bqs, 0.0, op0=ALU.add, op1=ALU.max)
    nc.vector.tensor_scalar(tmpa, ps0, bqs, 0.0, op0=ALU.add, op1=ALU.min)
    nc.vector.tensor_scalar(tmpb, tmpa, als, None, op0=ALU.mult)
    nc.vector.tensor_add(qp4[0:32, :], tmpr, tmpb)
    for j in range(1, 4):
        nc.sync.dma_start(out=qp4[32 * j:32 * j + 32, :], in_=qp4[0:32, :])

    nc.vector.memset(negb, NEG_BIG)

    if K_BF16:
        wrep_b = const.tile([128, 3 * H1], BF16)
        qp4_b = const.tile([128, BC], BF16)
        nc.vector.tensor_copy(wrep_b, wrep.bitcast(F32))
        nc.vector.tensor_copy(qp4_b, qp4.bitcast(F32))
    else:
        wrep_b, qp4_b = wrep, qp4

    # persistent PSUM tiles. ps2 slots are padded to 64 so no matmul output
    # crosses a PSUM bank boundary; layout [128, j(t), c(blk), 64].
    ps1 = ps1p.tile([H1, 4, BC], F32, tag="ps1")       # 4 banks
    ps2A = ps2p.tile([128, 4, NBLK, 64], F32)          # 2 banks
    ps2B = ps2p.tile([128, 4, NBLK, 64], F32)          # 2 banks
    # pad columns are never matmul-written; zero once so tanh(junk) can't NaN
    nc.vector.memset(ps2A, 0.0)
    nc.vector.memset(ps2B, 0.0)

    for _rep in range(repeat):
        _main_pass(nc, d, TT, NG, ktp, qkp, a2p, prp, ps1, (ps2A, ps2B),
                   (a1A, a1B), logt, wrep_b, w2aug, wfb, b1h, qp4_b, mki, negb,
                   mx, sums, rin, expv, att)


def _main_pass(nc, d, TT, NG, ktp, qkp, a2p, prp, ps1, ps2s, a1s_ab, logt,
               wrep, w2aug, wfb, b1h, qp4, mki, negb, mx, sums, rin, expv, att):
    kts, qks, a2s = {}, {}, {}

    KD = BF16 if K_BF16 else F32R

    def dma_kt(g):
        kt = ktp.tile([128, BC], KD)
        nc.sync.dma_start(
            out=kt, in_=d["kT"][4 * g:4 * g + 4].rearrange("tj f b -> (tj f) b"))
        kts[g] = kt

    def emit_qk(g):
        qk = qkp.tile([128, BC], KD)
        nc.vector.tensor_mul(qk, qp4, kts[g])
        qks[g] = qk

    def emit_l1(g):
        kt, qk = kts.pop(g), qks.pop(g)
        for j in range(4):
            s = slice(32 * j, 32 * j + 32)
            tp = (32 * j, 0)
            p1 = ps1[:, j, :]
            nc.tensor.matmul(p1, wrep[s, 0:H1], qp4[s, :], start=True,
                             stop=False, tile_position=tp)
            nc.tensor.matmul(p1, wrep[s, H1:2 * H1], kt[s, :], start=False,
                             stop=False, tile_position=tp)
            nc.tensor.matmul(p1, wrep[s, 2 * H1:3 * H1], qk[s, :], start=False,
                             stop=True, tile_position=tp)

    def emit_s1(g):
        a1 = a1s_ab[g % 2]
        nc.scalar.activation(a1[0:H1, :, :], ps1, AF.Tanh, bias=b1h, scale=0.5)

    def emit_l2(g):
        a1 = a1s_ab[g % 2]
        ps2 = ps2s[g % 2]
        for j in range(4):
            for c in range(NBLK):
                nc.tensor.matmul(
                    ps2[:, j, c, 0:H2],
                    a1[:, j, 128 * c:128 * c + 128],
                    w2aug,
                    start=True,
                    stop=True,
                )

    def emit_s2(g):
        ps2 = ps2s[g % 2]
        a2 = a2p.tile([128, 4, NBLK, 64], BF16)
        if S2_PADDED:
            nc.scalar.activation(a2, ps2, AF.Tanh, scale=0.5)
        else:
            nc.scalar.activation(a2[:, :, :, 0:H2], ps2[:, :, :, 0:H2], AF.Tanh,
                                 scale=0.5)
        a2s[g] = a2

    def emit_logits(g):
        a2 = a2s.pop(g)
        pr = prp.tile([128, 4, NBLK, 64], BF16)
        nc.vector.tensor_mul(
            pr[:, :, :, 0:H2], a2[:, :, :, 0:H2],
            wfb.unsqueeze(1).unsqueeze(1).broadcast_to([128, 4, NBLK, H2]))
        # out view [128, j, c] of logt[:, c, t0+j] via free-dim transpose
        nc.vector.tensor_reduce(
            logt[:, :, 4 * g:4 * g + 4].transpose([0, 2, 1]),
            pr[:, :, :, 0:H2], axis=AX.X, op=ALU.add)

    # ---- software-pipelined main loop ----
    dma_kt(0)
    emit_qk(0)
    emit_l1(0)
    emit_s1(0)
    for g in range(NG):
        if g + 1 < NG:
            dma_kt(g + 1)
            emit_qk(g + 1)
            emit_l1(g + 1)
        emit_l2(g)
        if g >= 1:
            emit_logits(g - 1)
        if g + 1 < NG:
            emit_s1(g + 1)
        emit_s2(g)
    emit_logits(NG - 1)

    # ---- masked softmax over t ----
    # Per-block pipeline: exp (ACT) || sum+recip+scale (DVE) || out-DMA of the
    # previous block. No ACT accum_out (its read-accumulator aux op is ~0.7us);
    # the sum runs on the otherwise-idle DVE instead.
    outv = d["out"].rearrange("(blk p) t -> p blk t", blk=NBLK)
    nc.vector.copy_predicated(logt, mki, negb)
    nc.vector.tensor_reduce(mx, logt, axis=AX.X, op=ALU.max, negate=True)
    if TAIL_PIPE:
        for blk in range(NBLK):
            nc.scalar.activation(
                expv[:, blk, :],
                logt[:, blk, :],
                AF.Exp,
                bias=mx[:, blk:blk + 1],
            )
            nc.vector.tensor_reduce(
                sums[:, blk:blk + 1], expv[:, blk, :], axis=AX.X, op=ALU.add)
            nc.vector.reciprocal(rin[:, blk:blk + 1], sums[:, blk:blk + 1])
            nc.vector.tensor_scalar(
                att[:, blk, :], expv[:, blk, :], rin[:, blk:blk + 1], None,
                op0=ALU.mult)
            nc.sync.dma_start(out=outv[:, blk, :], in_=att[:, blk, :])
    else:
        for blk in range(NBLK):
            nc.scalar.activation(
                expv[:, blk, :], logt[:, blk, :], AF.Exp,
                bias=mx[:, blk:blk + 1], accum_out=sums[:, blk:blk + 1])
        nc.vector.reciprocal(rin, sums)
        for blk in range(NBLK):
            nc.vector.tensor_scalar(
                att[:, blk, :], expv[:, blk, :], rin[:, blk:blk + 1], None,
                op0=ALU.mult)
        nc.sync.dma_start(out=outv, in_=att)


def build(TT=T, repeat=1):
    nc = bacc.Bacc("TRN2", target_bir_lowering=False, debug=False,
                   num_devices=N_CORES)
    d = {
        "kT": nc.dram_tensor("kT", [TT, D, BC], BF16 if K_BF16 else F32R,
                             kind="ExternalInput").ap(),
        "qT": nc.dram_tensor("qT", [D, BC], F32R, kind="ExternalInput").ap(),
        "mki": nc.dram_tensor("mki", [128, NBLK, TT], I8, kind="ExternalInput").ap(),
        "Wq": nc.dram_tensor("Wq", [D, D], F32R, kind="ExternalInput").ap(),
        "bq": nc.dram_tensor("bq", [D, 1], F32, kind="ExternalInput").ap(),
        "alpha": nc.dram_tensor("alpha", [D, 1], F32, kind="ExternalInput").ap(),
        "W1": nc.dram_tensor("W1", [4 * D, H1], F32R, kind="ExternalInput").ap(),
        "b1h": nc.dram_tensor("b1h", [H1, 1], F32, kind="ExternalInput").ap(),
        "W2aug": nc.dram_tensor("W2aug", [H1 + 1, H2], F32,
                                kind="ExternalInput").ap(),
        "wfb": nc.dram_tensor("wfb", [128, H2], F32, kind="ExternalInput").ap(),
        "out": nc.dram_tensor("out", [BC, TT], F32, kind="ExternalOutput").ap(),
    }
    with tile.TileContext(nc) as tc:
        with contextlib.ExitStack() as es:
            _emit(nc, tc, es, d, TT, repeat=repeat)
    nc.compile()
    return nc


def prepare(q, k, mask, Wq, bq, alpha, W1, b1, W2, b2, Wf, bf=None):
    """Varlen packing: per batch row keep only its unmasked t's (plus padding to
    the global max count, rounded to a multiple of 4). Pure index manipulation.
    Returns (in_maps, TT, tidx)."""
    mask_np = np.asarray(mask)
    cnt = (mask_np != 0).sum(1)                      # unmasked count per row
    if cnt.min() == 0:
        # a fully-masked row needs the uniform-softmax semantics; identity
        # "compaction" reproduces the uncompacted kernel exactly
        TT = T
        tidx = np.ascontiguousarray(np.tile(np.arange(T), (B, 1)))
        pad = mask_np == 0
    else:
        TT = int(-(-int(cnt.max()) // 4) * 4)        # round up to x4
        TT = max(TT, 8)
        order = np.argsort(mask_np == 0, axis=1, kind="stable")  # unmasked first
        tidx = np.ascontiguousarray(order[:, :TT])   # [B, TT]
        pad = (np.arange(TT)[None, :] >= cnt[:, None])  # True on pad slots
    kc = np.take_along_axis(np.asarray(k), tidx[:, :, None], axis=1)  # [B, TT, D]

    W2 = np.asarray(W2, np.float32)
    b2 = np.asarray(b2, np.float32)
    Wf = np.asarray(Wf, np.float32).reshape(H2)
    b1 = np.asarray(b1, np.float32)
    # sigma(x) = 0.5 + 0.5*tanh(x/2) folding:
    #   z2 = W2^T sig1 + b2 = (0.5 W2)^T t1 + (b2 + 0.5 * sum_h W2[h,:])
    #   logit = Wf . sig2 (+const) = (0.5 Wf) . t2 + const   (const drops in softmax)
    c2 = b2 + 0.5 * W2.sum(axis=0)                   # [H2]
    W2aug = np.concatenate([0.5 * W2, c2[None, :]], axis=0)  # [81, 40]
    wfb = np.tile((0.5 * Wf)[None, :], (128, 1))     # [128, 40]

    common = {
        "Wq": np.ascontiguousarray(Wq, np.float32),
        "bq": np.ascontiguousarray(bq, np.float32).reshape(D, 1),
        "alpha": np.ascontiguousarray(alpha, np.float32).reshape(D, 1),
        "W1": np.ascontiguousarray(W1, np.float32),
        "b1h": np.ascontiguousarray(0.5 * b1, np.float32).reshape(H1, 1),
        "W2aug": np.ascontiguousarray(W2aug, np.float32),
        "wfb": np.ascontiguousarray(wfb, np.float32),
    }
    in_maps = []
    for c in range(N_CORES):
        sl = slice(c * BC, (c + 1) * BC)
        if K_BF16:
            import ml_dtypes
            kcc = np.ascontiguousarray(
                kc[sl].transpose(1, 2, 0).astype(ml_dtypes.bfloat16))
        else:
            kcc = np.ascontiguousarray(kc[sl].transpose(1, 2, 0), np.float32)
        qc = np.ascontiguousarray(np.asarray(q)[sl].T, np.float32)  # [D, BC]
        mc = pad[sl].astype(np.int8)
        mc = np.ascontiguousarray(mc.reshape(NBLK, 128, TT).transpose(1, 0, 2))
        m = dict(common)
        m.update({"kT": kcc, "qT": qc, "mki": mc})
        in_maps.append(m)
    return in_maps, TT, tidx


def postprocess(results, TT, tidx):
    attc = np.empty((B, TT), np.float32)
    for c in range(N_CORES):
        attc[c * BC:(c + 1) * BC] = results[c]["out"]
    out = np.zeros((B, T), np.float32)
    np.put_along_axis(out, tidx, attc, axis=1)
    return out.reshape(B, 1, T)


_NC_CACHE = {}


def kernel(**inputs):
    in_maps, TT, tidx = prepare(
        inputs["q"], inputs["k"], inputs["mask"], inputs["Wq"], inputs["bq"],
        inputs["alpha"], inputs["W1"], inputs["b1"], inputs["W2"], inputs["b2"],
        inputs["Wf"],
    )
    if TT not in _NC_CACHE:
        _NC_CACHE[TT] = build(TT=TT)
    nc = _NC_CACHE[TT]
    res = run_bass_kernel_spmd(nc, in_maps, core_ids=list(range(N_CORES)))
    return postprocess(res.results, TT, tidx)


# revision 33
# speedup vs baseline: 1.2453x; 1.1968x over previous
"""DIN-style attention (MLP over [qt, k, qt-k, qt*k] + masked softmax) on 8 TRN2 cores.

Data-parallel over batch: each core handles 512 of 4096 rows.

Structure (v3, ACT-bound design with transposed layer 2):
  - sigmoid -> tanh identity: sigma(x) = 0.5 + 0.5*tanh(x/2). The 0.5 factors fold
    into W2, b2, Wf on the host; the constant logit shift cancels in softmax.
    Tanh and Exp share one activation table set -> no table swaps, ever.
  - W1 is algebraically combined on-device: info@W1 = qp@(W1q+W1m) + k@(W1k-W1m)
    + (qp*k)@W1p, so the (qt-k) features never materialize.
  - L1: per group of 4 t's, 12 K=32 matmuls at 4 distinct PE row strips (run
    concurrently on HW) into a 4-bank PSUM tile [80, 4, 512]; sigma1 is ONE tanh
    per 4 t's (free dim 2048, 80 lanes).
  - L2 is TRANSPOSED: stationary = bf16 a1 chunk [81, 128] (row 80 = ones, which
    delivers the bias via W2aug's last row), moving = W2aug [81, 40]. Output is
    [128 b-partitions, 40 h2-free] -> sigma2 runs at FULL 128 lanes: one tanh per
    4 t's at free dim 640 (4x cheaper than feature-major). bf16 stationary
    triggers Fast Weight Load (128 cols).
  - L3 matmuls are GONE: logit[b,t] = sum_h wf_h * a2[b,t,h] is a DVE
    multiply + segmented reduce straight into the [128 b, t] softmax layout
    (SBUF, no PSUM needed).
  - Emission is software-pipelined so ACT (the bottleneck engine) never stalls.
"""

import contextlib

import numpy as np

import concourse.bacc as bacc
import concourse.mybir as mybir
import concourse.tile as tile
from concourse.bass_utils import run_bass_kernel_spmd

N_CORES = 8
B, T, D = 4096, 200, 32
BC = B // N_CORES  # 512 rows per core
H1, H2 = 80, 40
NBLK = BC // 128   # 4 blocks of 128 b's
NEG_BIG = float(np.float32(-2.0**32 + 1.0))

S2_PADDED = True   # sigma2 reads the full padded [.,1024] span vs strided 640
S2_TIGHT = False   # 40-stride ps2 slots (bank-edge slot split in two) -> 640 span
TAIL_PIPE = True   # per-block softmax tail (DVE sums) vs monolithic ACT accum
K_BF16 = True      # carry k / qp / qk / W1 through the L1 matmuls in bf16

F32 = mybir.dt.float32
F32R = mybir.dt.float32r
BF16 = mybir.dt.bfloat16
I8 = mybir.dt.int8
AF = mybir.ActivationFunctionType
ALU = mybir.AluOpType
AX = mybir.AxisListType


def _emit(nc, tc, es, d, TT, repeat=1):
    NG = TT // 4
    const = es.enter_context(tc.tile_pool(name="const", bufs=1))
    ktp = es.enter_context(tc.tile_pool(name="ktp", bufs=4))
    qkp = es.enter_context(tc.tile_pool(name="qkp", bufs=3))
    a2p = es.enter_context(tc.tile_pool(name="a2p", bufs=2))
    prp = es.enter_context(tc.tile_pool(name="prp", bufs=2))
    ps1p = es.enter_context(tc.tile_pool(name="ps1p", bufs=1, space="PSUM"))
    ps2p = es.enter_context(tc.tile_pool(name="ps2p", bufs=1, space="PSUM"))

    # ---- static tiles ----
    w1raw = const.tile([4 * D, H1], F32R)
    wrep = const.tile([128, 3 * H1], F32R)  # strip j: [W1q+W1m | W1k-W1m | W1p]
    wq = const.tile([D, D], F32R)
    w2raw = const.tile([H1 + 1, H2], F32)
    w2aug = const.tile([H1 + 1, H2], BF16)  # [0.5*W2 ; c2] (bias via ones-row)
    wfraw = const.tile([128, H2], F32)
    wfb = const.tile([128, H2], BF16)       # 0.5*Wf replicated on all partitions
    b1h = const.tile([H1, 1], F32)          # b1 / 2
    bqs = const.tile([D, 1], F32)
    als = const.tile([D, 1], F32)
    qts = const.tile([D, BC], F32R)
    qp4 = const.tile([128, BC], F32R)       # qp^T replicated at 4 strips
    mki = const.tile([128, NBLK, TT], I8)
    negb = const.tile([128, NBLK, TT], F32)
    tmpr = const.tile([D, BC], F32)
    tmpa = const.tile([D, BC], F32)
    tmpb = const.tile([D, BC], F32)
    # a1 double buffer: [81, 4, BC] bf16, row 80 = ones (bias row)
    a1A = const.tile([H1 + 1, 4, BC], BF16)
    a1B = const.tile([H1 + 1, 4, BC], BF16)
    logt = const.tile([128, NBLK, TT], F32)
    mx = const.tile([128, NBLK], F32)
    sums = const.tile([128, NBLK], F32)
    rin = const.tile([128, NBLK], F32)
    expv = const.tile([128, NBLK, TT], F32)
    att = const.tile([128, NBLK, TT], F32)

    nc.sync.dma_start(out=w1raw, in_=d["W1"])
    nc.sync.dma_start(out=wq, in_=d["Wq"])
    nc.sync.dma_start(out=w2raw, in_=d["W2aug"])
    nc.sync.dma_start(out=wfraw, in_=d["wfb"])
    nc.vector.tensor_copy(w2aug, w2raw)
    nc.vector.tensor_copy(wfb, wfraw)
    nc.sync.dma_start(out=b1h, in_=d["b1h"])
    nc.sync.dma_start(out=bqs, in_=d["bq"])
    nc.sync.dma_start(out=als, in_=d["alpha"])
    nc.sync.dma_start(out=qts, in_=d["qT"])
    nc.sync.dma_start(out=mki, in_=d["mki"])
    # ones bias-row at partition 80: DVE can't start mid-strip -> memset at
    # partition 0 and DMA the row into place
    ones1 = const.tile([1, 4 * BC], BF16)
    nc.vector.memset(ones1, 1.0)
    nc.sync.dma_start(
        out=a1A[H1:H1 + 1, :, :], in_=ones1.rearrange("p (j b) -> p j b", j=4))
    nc.sync.dma_start(
        out=a1B[H1:H1 + 1, :, :], in_=ones1.rearrange("p (j b) -> p j b", j=4))

    # dummy tanh: hoists the activation-table load (exp_and_others covers both
    # Tanh and Exp) into setup so the first real tanh doesn't serialize on it
    nc.scalar.activation(tmpr[:, 0:1], bqs, AF.Tanh)

    # combined W1 blocks, then replicate to strips 1..3.
    # HW verifier (NCC_IBIR297) requires equal base partitions for 2-input DVE
    # ops, so align the blocks to base 0 first; cross-partition moves go via DMA.
    t32 = const.tile([32, H1], F32R)
    t64 = const.tile([32, H1], F32R)
    nc.sync.dma_start(out=t32, in_=d["W1"][32:64, :])
    nc.sync.dma_start(out=t64, in_=d["W1"][64:96, :])
    nc.vector.tensor_add(wrep[0:32, 0:H1], w1raw[0:32, :], t64)
    nc.vector.tensor_sub(wrep[0:32, H1:2 * H1], t32, t64)
    nc.sync.dma_start(out=wrep[0:32, 2 * H1:3 * H1], in_=d["W1"][96:128, :])
    for j in range(1, 4):
        nc.sync.dma_start(out=wrep[32 * j:32 * j + 32, :], in_=wrep[0:32, :])

    # qp^T = prelu(Wq^T @ q^T + bq, alpha)
    ps0 = ps1p.tile([D, BC], F32, tag="ps1")
    nc.tensor.matmul(ps0, wq, qts, start=True, stop=True)
    nc.vector.tensor_scalar(tmpr, ps0, bqs, 0.0, op0=ALU.add, op1=ALU.max)
    nc.vector.tensor_scalar(tmpa, ps0, bqs, 0.0, op0=ALU.add, op1=ALU.min)
    nc.vector.tensor_scalar(tmpb, tmpa, als, None, op0=ALU.mult)
    nc.vector.tensor_add(qp4[0:32, :], tmpr, tmpb)
    for j in range(1, 4):
        nc.sync.dma_start(out=qp4[32 * j:32 * j + 32, :], in_=qp4[0:32, :])

    nc.vector.memset(negb, NEG_BIG)

    if K_BF16:
        wrep_b = const.tile([128, 3 * H1], BF16)
        qp4_b = const.tile([128, BC], BF16)
        nc.vector.tensor_copy(wrep_b, wrep.bitcast(F32))
        nc.vector.tensor_copy(qp4_b, qp4.bitcast(F32))
    else:
        wrep_b, qp4_b = wrep, qp4

    # persistent PSUM tiles. Matmul outputs must not cross a PSUM bank boundary:
    # either pad slots to 64 ([128, j, c, 64]) or pack slots at 40 and split the
    # one bank-edge-crossing slot's matmul in two ([128, 16, 40], S2_TIGHT).
    ps1 = ps1p.tile([H1, 4, BC], F32, tag="ps1")       # 4 banks
    if S2_TIGHT:
        ps2A = ps2p.tile([128, 16, H2], F32)           # 640 f32 -> 2 banks
        ps2B = ps2p.tile([128, 16, H2], F32)
    else:
        ps2A = ps2p.tile([128, 4, NBLK, 64], F32)      # 2 banks
        ps2B = ps2p.tile([128, 4, NBLK, 64], F32)
    # (padded layout) pad columns are never matmul-written; zero once so
    # tanh(junk) can't NaN
    nc.vector.memset(ps2A, 0.0)
    nc.vector.memset(ps2B, 0.0)

    for _rep in range(repeat):
        _main_pass(nc, d, TT, NG, ktp, qkp, a2p, prp, ps1, (ps2A, ps2B),
                   (a1A, a1B), logt, wrep_b, w2aug, wfb, b1h, qp4_b, mki, negb,
                   mx, sums, rin, expv, att)


def _main_pass(nc, d, TT, NG, ktp, qkp, a2p, prp, ps1, ps2s, a1s_ab, logt,
               wrep, w2aug, wfb, b1h, qp4, mki, negb, mx, sums, rin, expv, att):
    kts, qks, a2s = {}, {}, {}

    KD = BF16 if K_BF16 else F32R

    def dma_kt(g):
        kt = ktp.tile([128, BC], KD)
        nc.sync.dma_start(
            out=kt, in_=d["kT"][4 * g:4 * g + 4].rearrange("tj f b -> (tj f) b"))
        kts[g] = kt

    def emit_qk(g):
        qk = qkp.tile([128, BC], KD)
        nc.vector.tensor_mul(qk, qp4, kts[g])
        qks[g] = qk

    def emit_l1(g):
        kt, qk = kts.pop(g), qks.pop(g)
        for j in range(4):
            s = slice(32 * j, 32 * j + 32)
            tp = (32 * j, 0)
            p1 = ps1[:, j, :]
            nc.tensor.matmul(p1, wrep[s, 0:H1], qp4[s, :], start=True,
                             stop=False, tile_position=tp)
            nc.tensor.matmul(p1, wrep[s, H1:2 * H1], kt[s, :], start=False,
                             stop=False, tile_position=tp)
            nc.tensor.matmul(p1, wrep[s, 2 * H1:3 * H1], qk[s, :], start=False,
                             stop=True, tile_position=tp)

    def emit_s1(g):
        a1 = a1s_ab[g % 2]
        nc.scalar.activation(a1[0:H1, :, :], ps1, AF.Tanh, bias=b1h, scale=0.5)

    def emit_l2(g):
        a1 = a1s_ab[g % 2]
        ps2 = ps2s[g % 2]
        for j in range(4):
            for c in range(NBLK):
                lhsT = a1[:, j, 128 * c:128 * c + 128]
                if not S2_TIGHT:
                    nc.tensor.matmul(ps2[:, j, c, 0:H2], lhsT, w2aug,
                                     start=True, stop=True)
                    continue
                s = 4 * j + c
                lo = s * H2
                cut = 512 - lo  # elements of this slot left in the lower bank
                if 0 < cut < H2:
                    nc.tensor.matmul(ps2[:, s, 0:cut], lhsT, w2aug[:, 0:cut],
                                     start=True, stop=True)
                    nc.tensor.matmul(ps2[:, s, cut:H2], lhsT, w2aug[:, cut:H2],
                                     start=True, stop=True)
                else:
                    nc.tensor.matmul(ps2[:, s, :], lhsT, w2aug,
                                     start=True, stop=True)

    def emit_s2(g):
        ps2 = ps2s[g % 2]
        if S2_TIGHT:
            a2 = a2p.tile([128, 16, H2], BF16)
            nc.scalar.activation(a2, ps2, AF.Tanh, scale=0.5)
        else:
            a2 = a2p.tile([128, 4, NBLK, 64], BF16)
            if S2_PADDED:
                nc.scalar.activation(a2, ps2, AF.Tanh, scale=0.5)
            else:
                nc.scalar.activation(a2[:, :, :, 0:H2], ps2[:, :, :, 0:H2],
                                     AF.Tanh, scale=0.5)
        a2s[g] = a2

    def emit_logits(g):
        a2 = a2s.pop(g)
        if S2_TIGHT:
            pr = prp.tile([128, 16, H2], BF16)
            nc.vector.tensor_mul(
                pr, a2, wfb.unsqueeze(1).broadcast_to([128, 16, H2]))
            prv = pr.rearrange("p (j c) h -> p j c h", j=4)
        else:
            pr = prp.tile([128, 4, NBLK, 64], BF16)
            nc.vector.tensor_mul(
                pr[:, :, :, 0:H2], a2[:, :, :, 0:H2],
                wfb.unsqueeze(1).unsqueeze(1).broadcast_to([128, 4, NBLK, H2]))
            prv = pr[:, :, :, 0:H2]
        # out view [128, j, c] of logt[:, c, t0+j] via free-dim transpose
        nc.vector.tensor_reduce(
            logt[:, :, 4 * g:4 * g + 4].transpose([0, 2, 1]),
            prv, axis=AX.X, op=ALU.add)

    # ---- software-pipelined main loop ----
    dma_kt(0)
    emit_qk(0)
    emit_l1(0)
    emit_s1(0)
    for g in range(NG):
        if g + 1 < NG:
            dma_kt(g + 1)
            emit_qk(g + 1)
            emit_l1(g + 1)
        emit_l2(g)
        if g >= 1:
            emit_logits(g - 1)
        if g + 1 < NG:
            emit_s1(g + 1)
        emit_s2(g)
    emit_logits(NG - 1)

    # ---- masked softmax over t ----
    # Per-block pipeline: exp (ACT) || sum+recip+scale (DVE) || out-DMA of the
    # previous block. No ACT accum_out (its read-accumulator aux op is ~0.7us);
    # the sum runs on the otherwise-idle DVE instead.
    outv = d["out"].rearrange("(blk p) t -> p blk t", blk=NBLK)
    nc.vector.copy_predicated(logt, mki, negb)
    nc.vector.tensor_reduce(mx, logt, axis=AX.X, op=ALU.max, negate=True)
    if TAIL_PIPE:
        for blk in range(NBLK):
            nc.scalar.activation(
                expv[:, blk, :],
                logt[:, blk, :],
                AF.Exp,
                bias=mx[:, blk:blk + 1],
            )
            nc.vector.tensor_reduce(
                sums[:, blk:blk + 1], expv[:, blk, :], axis=AX.X, op=ALU.add)
            nc.vector.reciprocal(rin[:, blk:blk + 1], sums[:, blk:blk + 1])
            nc.vector.tensor_scalar(
                att[:, blk, :], expv[:, blk, :], rin[:, blk:blk + 1], None,
                op0=ALU.mult)
            nc.sync.dma_start(out=outv[:, blk, :], in_=att[:, blk, :])
    else:
        for blk in range(NBLK):
            nc.scalar.activation(
                expv[:, blk, :], logt[:, blk, :], AF.Exp,
                bias=mx[:, blk:blk + 1], accum_out=sums[:, blk:blk + 1])
        nc.vector.reciprocal(rin, sums)
        for blk in range(NBLK):
            nc.vector.tensor_scalar(
                att[:, blk, :], expv[:, blk, :], rin[:, blk:blk + 1], None,
                op0=ALU.mult)
        nc.sync.dma_start(out=outv, in_=att)


def build(TT=T, repeat=1):
    nc = bacc.Bacc("TRN2", target_bir_lowering=False, debug=False,
                   num_devices=N_CORES)
    d = {
        "kT": nc.dram_tensor("kT", [TT, D, BC], BF16 if K_BF16 else F32R,
                             kind="ExternalInput").ap(),
        "qT": nc.dram_tensor("qT", [D, BC], F32R, kind="ExternalInput").ap(),
        "mki": nc.dram_tensor("mki", [128, NBLK, TT], I8, kind="ExternalInput").ap(),
        "Wq": nc.dram_tensor("Wq", [D, D], F32R, kind="ExternalInput").ap(),
        "bq": nc.dram_tensor("bq", [D, 1], F32, kind="ExternalInput").ap(),
        "alpha": nc.dram_tensor("alpha", [D, 1], F32, kind="ExternalInput").ap(),
        "W1": nc.dram_tensor("W1", [4 * D, H1], F32R, kind="ExternalInput").ap(),
        "b1h": nc.dram_tensor("b1h", [H1, 1], F32, kind="ExternalInput").ap(),
        "W2aug": nc.dram_tensor("W2aug", [H1 + 1, H2], F32,
                                kind="ExternalInput").ap(),
        "wfb": nc.dram_tensor("wfb", [128, H2], F32, kind="ExternalInput").ap(),
        "out": nc.dram_tensor("out", [BC, TT], F32, kind="ExternalOutput").ap(),
    }
    with tile.TileContext(nc) as tc:
        with contextlib.ExitStack() as es:
            _emit(nc, tc, es, d, TT, repeat=repeat)
    nc.compile()
    return nc


def prepare(q, k, mask, Wq, bq, alpha, W1, b1, W2, b2, Wf, bf=None):
    """Varlen packing: per batch row keep only its unmasked t's (plus padding to
    the global max count, rounded to a multiple of 4). Pure index manipulation.
    Returns (in_maps, TT, tidx)."""
    mask_np = np.asarray(mask)
    cnt = (mask_np != 0).sum(1)                      # unmasked count per row
    if cnt.min() == 0:
        # a fully-masked row needs the uniform-softmax semantics; identity
        # "compaction" reproduces the uncompacted kernel exactly
        TT = T
        tidx = np.ascontiguousarray(np.tile(np.arange(T), (B, 1)))
        pad = mask_np == 0
    else:
        TT = int(-(-int(cnt.max()) // 4) * 4)        # round up to x4
        TT = max(TT, 8)
        order = np.argsort(mask_np == 0, axis=1, kind="stable")  # unmasked first
        tidx = np.ascontiguousarray(order[:, :TT])   # [B, TT]
        pad = (np.arange(TT)[None, :] >= cnt[:, None])  # True on pad slots
    kc = np.take_along_axis(np.asarray(k), tidx[:, :, None], axis=1)  # [B, TT, D]

    W2 = np.asarray(W2, np.float32)
    b2 = np.asarray(b2, np.float32)
    Wf = np.asarray(Wf, np.float32).reshape(H2)
    b1 = np.asarray(b1, np.float32)
    # sigma(x) = 0.5 + 0.5*tanh(x/2) folding:
    #   z2 = W2^T sig1 + b2 = (0.5 W2)^T t1 + (b2 + 0.5 * sum_h W2[h,:])
    #   logit = Wf . sig2 (+const) = (0.5 Wf) . t2 + const   (const drops in softmax)
    c2 = b2 + 0.5 * W2.sum(axis=0)                   # [H2]
    W2aug = np.concatenate([0.5 * W2, c2[None, :]], axis=0)  # [81, 40]
    wfb = np.tile((0.5 * Wf)[None, :], (128, 1))     # [128, 40]

    common = {
        "Wq": np.ascontiguousarray(Wq, np.float32),
        "bq": np.ascontiguousarray(bq, np.float32).reshape(D, 1),
        "alpha": np.ascontiguousarray(alpha, np.float32).reshape(D, 1),
        "W1": np.ascontiguousarray(W1, np.float32),
        "b1h": np.ascontiguousarray(0.5 * b1, np.float32).reshape(H1, 1),
        "W2aug": np.ascontiguousarray(W2aug, np.float32),
        "wfb": np.ascontiguousarray(wfb, np.float32),
    }
    in_maps = []
    for c in range(N_CORES):
        sl = slice(c * BC, (c + 1) * BC)
        if K_BF16:
            import ml_dtypes
            kcc = np.ascontiguousarray(
                kc[sl].transpose(1, 2, 0).astype(ml_dtypes.bfloat16))
        else:
            kcc = np.ascontiguousarray(kc[sl].transpose(1, 2, 0), np.float32)
        qc = np.ascontiguousarray(np.asarray(q)[sl].T, np.float32)  # [D, BC]
        mc = pad[sl].astype(np.int8)
        mc = np.ascontiguousarray(mc.reshape(NBLK, 128, TT).transpose(1, 0, 2))
        m = dict(common)
        m.update({"kT": kcc, "qT": qc, "mki": mc})
        in_maps.append(m)
    return in_maps, TT, tidx


def postprocess(results, TT, tidx):
    attc = np.empty((B, TT), np.float32)
    for c in range(N_CORES):
        attc[c * BC:(c + 1) * BC] = results[c]["out"]
    out = np.zeros((B, T), np.float32)
    np.put_along_axis(out, tidx, attc, axis=1)
    return out.reshape(B, 1, T)


_NC_CACHE = {}


def kernel(**inputs):
    in_maps, TT, tidx = prepare(
        inputs["q"], inputs["k"], inputs["mask"], inputs["Wq"], inputs["bq"],
        inputs["alpha"], inputs["W1"], inputs["b1"], inputs["W2"], inputs["b2"],
        inputs["Wf"],
    )
    if TT not in _NC_CACHE:
        _NC_CACHE[TT] = build(TT=TT)
    nc = _NC_CACHE[TT]
    res = run_bass_kernel_spmd(nc, in_maps, core_ids=list(range(N_CORES)))
    return postprocess(res.results, TT, tidx)


# revision 34
# speedup vs baseline: 1.4107x; 1.1329x over previous
"""DIN-style attention (MLP over [qt, k, qt-k, qt*k] + masked softmax) on 8 TRN2 cores.

Data-parallel over batch: each core handles 512 of 4096 rows.

Structure (v3, ACT-bound design with transposed layer 2):
  - sigmoid -> tanh identity: sigma(x) = 0.5 + 0.5*tanh(x/2). The 0.5 factors fold
    into W2, b2, Wf on the host; the constant logit shift cancels in softmax.
    Tanh and Exp share one activation table set -> no table swaps, ever.
  - W1 is algebraically combined on-device: info@W1 = qp@(W1q+W1m) + k@(W1k-W1m)
    + (qp*k)@W1p, so the (qt-k) features never materialize.
  - L1: per group of 4 t's, 12 K=32 matmuls at 4 distinct PE row strips (run
    concurrently on HW) into a 4-bank PSUM tile [80, 4, 512]; sigma1 is ONE tanh
    per 4 t's (free dim 2048, 80 lanes).
  - L2 is TRANSPOSED: stationary = bf16 a1 chunk [81, 128] (row 80 = ones, which
    delivers the bias via W2aug's last row), moving = W2aug [81, 40]. Output is
    [128 b-partitions, 40 h2-free] -> sigma2 runs at FULL 128 lanes: one tanh per
    4 t's at free dim 640 (4x cheaper than feature-major). bf16 stationary
    triggers Fast Weight Load (128 cols).
  - L3 matmuls are GONE: logit[b,t] = sum_h wf_h * a2[b,t,h] is a DVE
    multiply + segmented reduce straight into the [128 b, t] softmax layout
    (SBUF, no PSUM needed).
  - Emission is software-pipelined so ACT (the bottleneck engine) never stalls.
"""

import contextlib

import numpy as np

import concourse.bacc as bacc
import concourse.mybir as mybir
import concourse.tile as tile
from concourse.bass_utils import run_bass_kernel_spmd

N_CORES = 8
B, T, D = 4096, 200, 32
BC = B // N_CORES  # 512 rows per core
H1, H2 = 80, 40
NBLK = BC // 128   # 4 blocks of 128 b's
NEG_BIG = float(np.float32(-2.0**32 + 1.0))

S2_PADDED = True   # sigma2 reads the full padded [.,1024] span vs strided 640
S2_TIGHT = True    # 40-stride ps2 slots (bank-edge slot split in two) -> 640 span
TAIL_PIPE = True   # per-block softmax tail (DVE sums) vs monolithic ACT accum
K_BF16 = True      # carry k / qp / qk / W1 through the L1 matmuls in bf16

F32 = mybir.dt.float32
F32R = mybir.dt.float32r
BF16 = mybir.dt.bfloat16
I8 = mybir.dt.int8
AF = mybir.ActivationFunctionType
ALU = mybir.AluOpType
AX = mybir.AxisListType


def _emit(nc, tc, es, d, TT, repeat=1):
    NG = TT // 4
    const = es.enter_context(tc.tile_pool(name="const", bufs=1))
    ktp = es.enter_context(tc.tile_pool(name="ktp", bufs=4))
    qkp = es.enter_context(tc.tile_pool(name="qkp", bufs=3))
    a2p = es.enter_context(tc.tile_pool(name="a2p", bufs=2))
    prp = es.enter_context(tc.tile_pool(name="prp", bufs=2))
    ps1p = es.enter_context(tc.tile_pool(name="ps1p", bufs=1, space="PSUM"))
    ps2p = es.enter_context(tc.tile_pool(name="ps2p", bufs=1, space="PSUM"))

    # ---- static tiles ----
    w1raw = const.tile([4 * D, H1], F32R)
    wrep = const.tile([128, 3 * H1], F32R)  # strip j: [W1q+W1m | W1k-W1m | W1p]
    wq = const.tile([D, D], F32R)
    w2raw = const.tile([H1 + 1, H2], F32)
    w2aug = const.tile([H1 + 1, H2], BF16)  # [0.5*W2 ; c2] (bias via ones-row)
    wfraw = const.tile([128, H2], F32)
    wfb = const.tile([128, H2], BF16)       # 0.5*Wf replicated on all partitions
    b1h = const.tile([H1, 1], F32)          # b1 / 2
    bqs = const.tile([D, 1], F32)
    als = const.tile([D, 1], F32)
    qts = const.tile([D, BC], F32R)
    qp4 = const.tile([128, BC], F32R)       # qp^T replicated at 4 strips
    mki = const.tile([128, NBLK, TT], I8)
    negb = const.tile([128, NBLK, TT], F32)
    tmpr = const.tile([D, BC], F32)
    tmpa = const.tile([D, BC], F32)
    tmpb = const.tile([D, BC], F32)
    # a1 double buffer: [81, 4, BC] bf16, row 80 = ones (bias row)
    a1A = const.tile([H1 + 1, 4, BC], BF16)
    a1B = const.tile([H1 + 1, 4, BC], BF16)
    logt = const.tile([128, NBLK, TT], F32)
    mx = const.tile([128, NBLK], F32)
    sums = const.tile([128, NBLK], F32)
    rin = const.tile([128, NBLK], F32)
    expv = const.tile([128, NBLK, TT], F32)
    att = const.tile([128, NBLK, TT], F32)

    nc.sync.dma_start(out=w1raw, in_=d["W1"])
    nc.sync.dma_start(out=wq, in_=d["Wq"])
    nc.sync.dma_start(out=w2raw, in_=d["W2aug"])
    nc.sync.dma_start(out=wfraw, in_=d["wfb"])
    nc.vector.tensor_copy(w2aug, w2raw)
    nc.vector.tensor_copy(wfb, wfraw)
    nc.sync.dma_start(out=b1h, in_=d["b1h"])
    nc.sync.dma_start(out=bqs, in_=d["bq"])
    nc.sync.dma_start(out=als, in_=d["alpha"])
    nc.sync.dma_start(out=qts, in_=d["qT"])
    nc.sync.dma_start(out=mki, in_=d["mki"])
    # ones bias-row at partition 80: DVE can't start mid-strip -> memset at
    # partition 0 and DMA the row into place
    ones1 = const.tile([1, 4 * BC], BF16)
    nc.vector.memset(ones1, 1.0)
    nc.sync.dma_start(
        out=a1A[H1:H1 + 1, :, :], in_=ones1.rearrange("p (j b) -> p j b", j=4))
    nc.sync.dma_start(
        out=a1B[H1:H1 + 1, :, :], in_=ones1.rearrange("p (j b) -> p j b", j=4))

    # dummy tanh: hoists the activation-table load (exp_and_others covers both
    # Tanh and Exp) into setup so the first real tanh doesn't serialize on it
    nc.scalar.activation(tmpr[:, 0:1], bqs, AF.Tanh)

    # combined W1 blocks, then replicate to strips 1..3.
    # HW verifier (NCC_IBIR297) requires equal base partitions for 2-input DVE
    # ops, so align the blocks to base 0 first; cross-partition moves go via DMA.
    t32 = const.tile([32, H1], F32R)
    t64 = const.tile([32, H1], F32R)
    nc.sync.dma_start(out=t32, in_=d["W1"][32:64, :])
    nc.sync.dma_start(out=t64, in_=d["W1"][64:96, :])
    nc.vector.tensor_add(wrep[0:32, 0:H1], w1raw[0:32, :], t64)
    nc.vector.tensor_sub(wrep[0:32, H1:2 * H1], t32, t64)
    nc.sync.dma_start(out=wrep[0:32, 2 * H1:3 * H1], in_=d["W1"][96:128, :])
    for j in range(1, 4):
        nc.sync.dma_start(out=wrep[32 * j:32 * j + 32, :], in_=wrep[0:32, :])

    # qp^T = prelu(Wq^T @ q^T + bq, alpha)
    ps0 = ps1p.tile([D, BC], F32, tag="ps1")
    nc.tensor.matmul(ps0, wq, qts, start=True, stop=True)
    nc.vector.tensor_scalar(tmpr, ps0, bqs, 0.0, op0=ALU.add, op1=ALU.max)
    nc.vector.tensor_scalar(tmpa, ps0, bqs, 0.0, op0=ALU.add, op1=ALU.min)
    nc.vector.tensor_scalar(tmpb, tmpa, als, None, op0=ALU.mult)
    nc.vector.tensor_add(qp4[0:32, :], tmpr, tmpb)
    for j in range(1, 4):
        nc.sync.dma_start(out=qp4[32 * j:32 * j + 32, :], in_=qp4[0:32, :])

    nc.vector.memset(negb, NEG_BIG)

    if K_BF16:
        wrep_b = const.tile([128, 3 * H1], BF16)
        qp4_b = const.tile([128, BC], BF16)
        nc.vector.tensor_copy(wrep_b, wrep.bitcast(F32))
        nc.vector.tensor_copy(qp4_b, qp4.bitcast(F32))
    else:
        wrep_b, qp4_b = wrep, qp4

    # persistent PSUM tiles. Matmul outputs must not cross a PSUM bank boundary:
    # either pad slots to 64 ([128, j, c, 64]) or pack slots at 40 and split the
    # one bank-edge-crossing slot's matmul in two ([128, 16, 40], S2_TIGHT).
    ps1 = ps1p.tile([H1, 4, BC], F32, tag="ps1")       # 4 banks
    if S2_TIGHT:
        ps2A = ps2p.tile([128, 16, H2], F32)           # 640 f32 -> 2 banks
        ps2B = ps2p.tile([128, 16, H2], F32)
    else:
        ps2A = ps2p.tile([128, 4, NBLK, 64], F32)      # 2 banks
        ps2B = ps2p.tile([128, 4, NBLK, 64], F32)
    # (padded layout) pad columns are never matmul-written; zero once so
    # tanh(junk) can't NaN
    nc.vector.memset(ps2A, 0.0)
    nc.vector.memset(ps2B, 0.0)

    for _rep in range(repeat):
        _main_pass(nc, d, TT, NG, ktp, qkp, a2p, prp, ps1, (ps2A, ps2B),
                   (a1A, a1B), logt, wrep_b, w2aug, wfb, b1h, qp4_b, mki, negb,
                   mx, sums, rin, expv, att)


def _main_pass(nc, d, TT, NG, ktp, qkp, a2p, prp, ps1, ps2s, a1s_ab, logt,
               wrep, w2aug, wfb, b1h, qp4, mki, negb, mx, sums, rin, expv, att):
    kts, qks, a2s = {}, {}, {}

    KD = BF16 if K_BF16 else F32R

    def dma_kt(g):
        kt = ktp.tile([128, BC], KD)
        nc.sync.dma_start(
            out=kt, in_=d["kT"][4 * g:4 * g + 4].rearrange("tj f b -> (tj f) b"))
        kts[g] = kt

    def emit_qk(g):
        qk = qkp.tile([128, BC], KD)
        nc.vector.tensor_mul(qk, qp4, kts[g])
        qks[g] = qk

    def emit_l1(g):
        kt, qk = kts.pop(g), qks.pop(g)
        for j in range(4):
            s = slice(32 * j, 32 * j + 32)
            tp = (32 * j, 0)
            p1 = ps1[:, j, :]
            nc.tensor.matmul(p1, wrep[s, 0:H1], qp4[s, :], start=True,
                             stop=False, tile_position=tp)
            nc.tensor.matmul(p1, wrep[s, H1:2 * H1], kt[s, :], start=False,
                             stop=False, tile_position=tp)
            nc.tensor.matmul(p1, wrep[s, 2 * H1:3 * H1], qk[s, :], start=False,
                             stop=True, tile_position=tp)

    def emit_s1(g):
        a1 = a1s_ab[g % 2]
        nc.scalar.activation(a1[0:H1, :, :], ps1, AF.Tanh, bias=b1h, scale=0.5)

    def emit_l2(g):
        a1 = a1s_ab[g % 2]
        ps2 = ps2s[g % 2]
        for j in range(4):
            for c in range(NBLK):
                lhsT = a1[:, j, 128 * c:128 * c + 128]
                if not S2_TIGHT:
                    nc.tensor.matmul(ps2[:, j, c, 0:H2], lhsT, w2aug,
                                     start=True, stop=True)
                    continue
                s = 4 * j + c
                lo = s * H2
                cut = 512 - lo  # elements of this slot left in the lower bank
                if 0 < cut < H2:
                    nc.tensor.matmul(ps2[:, s, 0:cut], lhsT, w2aug[:, 0:cut],
                                     start=True, stop=True)
                    nc.tensor.matmul(ps2[:, s, cut:H2], lhsT, w2aug[:, cut:H2],
                                     start=True, stop=True)
                else:
                    nc.tensor.matmul(ps2[:, s, :], lhsT, w2aug,
                                     start=True, stop=True)

    def emit_s2(g):
        ps2 = ps2s[g % 2]
        if S2_TIGHT:
            a2 = a2p.tile([128, 16, H2], BF16)
            nc.scalar.activation(a2, ps2, AF.Tanh, scale=0.5)
        else:
            a2 = a2p.tile([128, 4, NBLK, 64], BF16)
            if S2_PADDED:
                nc.scalar.activation(a2, ps2, AF.Tanh, scale=0.5)
            else:
                nc.scalar.activation(a2[:, :, :, 0:H2], ps2[:, :, :, 0:H2],
                                     AF.Tanh, scale=0.5)
        a2s[g] = a2

    def emit_logits(g):
        a2 = a2s.pop(g)
        if S2_TIGHT:
            pr = prp.tile([128, 16, H2], BF16)
            nc.vector.tensor_mul(
                pr, a2, wfb.unsqueeze(1).broadcast_to([128, 16, H2]))
            prv = pr.rearrange("p (j c) h -> p j c h", j=4)
        else:
            pr = prp.tile([128, 4, NBLK, 64], BF16)
            nc.vector.tensor_mul(
                pr[:, :, :, 0:H2], a2[:, :, :, 0:H2],
                wfb.unsqueeze(1).unsqueeze(1).broadcast_to([128, 4, NBLK, H2]))
            prv = pr[:, :, :, 0:H2]
        # out view [128, j, c] of logt[:, c, t0+j] via free-dim transpose
        nc.vector.tensor_reduce(
            logt[:, :, 4 * g:4 * g + 4].transpose([0, 2, 1]),
            prv, axis=AX.X, op=ALU.add)

    # ---- software-pipelined main loop ----
    dma_kt(0)
    emit_qk(0)
    emit_l1(0)
    emit_s1(0)
    for g in range(NG):
        if g + 1 < NG:
            dma_kt(g + 1)
            emit_qk(g + 1)
            emit_l1(g + 1)
        emit_l2(g)
        if g >= 1:
            emit_logits(g - 1)
        if g + 1 < NG:
            emit_s1(g + 1)
        emit_s2(g)
    emit_logits(NG - 1)

    # ---- masked softmax over t ----
    # Per-block pipeline: exp (ACT) || sum+recip+scale (DVE) || out-DMA of the
    # previous block. No ACT accum_out (its read-accumulator aux op is ~0.7us);
    # the sum runs on the otherwise-idle DVE instead.
    outv = d["out"].rearrange("(blk p) t -> p blk t", blk=NBLK)
    nc.vector.copy_predicated(logt, mki, negb)
    nc.vector.tensor_reduce(mx, logt, axis=AX.X, op=ALU.max, negate=True)
    if TAIL_PIPE:
        for blk in range(NBLK):
            nc.scalar.activation(
                expv[:, blk, :],
                logt[:, blk, :],
                AF.Exp,
                bias=mx[:, blk:blk + 1],
            )
            nc.vector.tensor_reduce(
                sums[:, blk:blk + 1], expv[:, blk, :], axis=AX.X, op=ALU.add)
            nc.vector.reciprocal(rin[:, blk:blk + 1], sums[:, blk:blk + 1])
            nc.vector.tensor_scalar(
                att[:, blk, :], expv[:, blk, :], rin[:, blk:blk + 1], None,
                op0=ALU.mult)
            nc.sync.dma_start(out=outv[:, blk, :], in_=att[:, blk, :])
    else:
        for blk in range(NBLK):
            nc.scalar.activation(
                expv[:, blk, :], logt[:, blk, :], AF.Exp,
                bias=mx[:, blk:blk + 1], accum_out=sums[:, blk:blk + 1])
        nc.vector.reciprocal(rin, sums)
        for blk in range(NBLK):
            nc.vector.tensor_scalar(
                att[:, blk, :], expv[:, blk, :], rin[:, blk:blk + 1], None,
                op0=ALU.mult)
        nc.sync.dma_start(out=outv, in_=att)


def build(TT=T, repeat=1):
    nc = bacc.Bacc("TRN2", target_bir_lowering=False, debug=False,
                   num_devices=N_CORES)
    d = {
        "kT": nc.dram_tensor("kT", [TT, D, BC], BF16 if K_BF16 else F32R,
                             kind="ExternalInput").ap(),
        "qT": nc.dram_tensor("qT", [D, BC], F32R, kind="ExternalInput").ap(),
        "mki": nc.dram_tensor("mki", [128, NBLK, TT], I8, kind="ExternalInput").ap(),
        "Wq": nc.dram_tensor("Wq", [D, D], F32R, kind="ExternalInput").ap(),
        "bq": nc.dram_tensor("bq", [D, 1], F32, kind="ExternalInput").ap(),
        "alpha": nc.dram_tensor("alpha", [D, 1], F32, kind="ExternalInput").ap(),
        "W1": nc.dram_tensor("W1", [4 * D, H1], F32R, kind="ExternalInput").ap(),
        "b1h": nc.dram_tensor("b1h", [H1, 1], F32, kind="ExternalInput").ap(),
        "W2aug": nc.dram_tensor("W2aug", [H1 + 1, H2], F32,
                                kind="ExternalInput").ap(),
        "wfb": nc.dram_tensor("wfb", [128, H2], F32, kind="ExternalInput").ap(),
        "out": nc.dram_tensor("out", [BC, TT], F32, kind="ExternalOutput").ap(),
    }
    with tile.TileContext(nc) as tc:
        with contextlib.ExitStack() as es:
            _emit(nc, tc, es, d, TT, repeat=repeat)
    nc.compile()
    return nc


def prepare(q, k, mask, Wq, bq, alpha, W1, b1, W2, b2, Wf, bf=None):
    """Varlen packing: per batch row keep only its unmasked t's (plus padding to
    the global max count, rounded to a multiple of 4). Pure index manipulation.
    Returns (in_maps, TT, tidx)."""
    mask_np = np.asarray(mask)
    cnt = (mask_np != 0).sum(1)                      # unmasked count per row
    if cnt.min() == 0:
        # a fully-masked row needs the uniform-softmax semantics; identity
        # "compaction" reproduces the uncompacted kernel exactly
        TT = T
        tidx = np.ascontiguousarray(np.tile(np.arange(T), (B, 1)))
        pad = mask_np == 0
    else:
        TT = int(-(-int(cnt.max()) // 4) * 4)        # round up to x4
        TT = max(TT, 8)
        order = np.argsort(mask_np == 0, axis=1, kind="stable")  # unmasked first
        tidx = np.ascontiguousarray(order[:, :TT])   # [B, TT]
        pad = (np.arange(TT)[None, :] >= cnt[:, None])  # True on pad slots
    kc = np.take_along_axis(np.asarray(k), tidx[:, :, None], axis=1)  # [B, TT, D]

    W2 = np.asarray(W2, np.float32)
    b2 = np.asarray(b2, np.float32)
    Wf = np.asarray(Wf, np.float32).reshape(H2)
    b1 = np.asarray(b1, np.float32)
    # sigma(x) = 0.5 + 0.5*tanh(x/2) folding:
    #   z2 = W2^T sig1 + b2 = (0.5 W2)^T t1 + (b2 + 0.5 * sum_h W2[h,:])
    #   logit = Wf . sig2 (+const) = (0.5 Wf) . t2 + const   (const drops in softmax)
    c2 = b2 + 0.5 * W2.sum(axis=0)                   # [H2]
    W2aug = np.concatenate([0.5 * W2, c2[None, :]], axis=0)  # [81, 40]
    wfb = np.tile((0.5 * Wf)[None, :], (128, 1))     # [128, 40]

    common = {
        "Wq": np.ascontiguousarray(Wq, np.float32),
        "bq": np.ascontiguousarray(bq, np.float32).reshape(D, 1),
        "alpha": np.ascontiguousarray(alpha, np.float32).reshape(D, 1),
        "W1": np.ascontiguousarray(W1, np.float32),
        "b1h": np.ascontiguousarray(0.5 * b1, np.float32).reshape(H1, 1),
        "W2aug": np.ascontiguousarray(W2aug, np.float32),
        "wfb": np.ascontiguousarray(wfb, np.float32),
    }
    in_maps = []
    for c in range(N_CORES):
        sl = slice(c * BC, (c + 1) * BC)
        if K_BF16:
            import ml_dtypes
            kcc = np.ascontiguousarray(
                kc[sl].transpose(1, 2, 0).astype(ml_dtypes.bfloat16))
        else:
            kcc = np.ascontiguousarray(kc[sl].transpose(1, 2, 0), np.float32)
        qc = np.ascontiguousarray(np.asarray(q)[sl].T, np.float32)  # [D, BC]
        mc = pad[sl].astype(np.int8)
        mc = np.ascontiguousarray(mc.reshape(NBLK, 128, TT).transpose(1, 0, 2))
        m = dict(common)
        m.update({"kT": kcc, "qT": qc, "mki": mc})
        in_maps.append(m)
    return in_maps, TT, tidx


def postprocess(results, TT, tidx):
    attc = np.empty((B, TT), np.float32)
    for c in range(N_CORES):
        attc[c * BC:(c + 1) * BC] = results[c]["out"]
    out = np.zeros((B, T), np.float32)
    np.put_along_axis(out, tidx, attc, axis=1)
    return out.reshape(B, 1, T)


_NC_CACHE = {}


def kernel(**inputs):
    in_maps, TT, tidx = prepare(
        inputs["q"], inputs["k"], inputs["mask"], inputs["Wq"], inputs["bq"],
        inputs["alpha"], inputs["W1"], inputs["b1"], inputs["W2"], inputs["b2"],
        inputs["Wf"],
    )
    if TT not in _NC_CACHE:
        _NC_CACHE[TT] = build(TT=TT)
    nc = _NC_CACHE[TT]
    res = run_bass_kernel_spmd(nc, in_maps, core_ids=list(range(N_CORES)))
    return postprocess(res.results, TT, tidx)


# revision 35
# speedup vs baseline: 1.4422x; 1.0223x over previous
"""DIN-style attention (MLP over [qt, k, qt-k, qt*k] + masked softmax) on 8 TRN2 cores.

Data-parallel over batch: each core handles 512 of 4096 rows.

Structure (v3, ACT-bound design with transposed layer 2):
  - sigmoid -> tanh identity: sigma(x) = 0.5 + 0.5*tanh(x/2). The 0.5 factors fold
    into W2, b2, Wf on the host; the constant logit shift cancels in softmax.
    Tanh and Exp share one activation table set -> no table swaps, ever.
  - W1 is algebraically combined on-device: info@W1 = qp@(W1q+W1m) + k@(W1k-W1m)
    + (qp*k)@W1p, so the (qt-k) features never materialize.
  - L1: per group of 4 t's, 12 K=32 matmuls at 4 distinct PE row strips (run
    concurrently on HW) into a 4-bank PSUM tile [80, 4, 512]; sigma1 is ONE tanh
    per 4 t's (free dim 2048, 80 lanes).
  - L2 is TRANSPOSED: stationary = bf16 a1 chunk [81, 128] (row 80 = ones, which
    delivers the bias via W2aug's last row), moving = W2aug [81, 40]. Output is
    [128 b-partitions, 40 h2-free] -> sigma2 runs at FULL 128 lanes: one tanh per
    4 t's at free dim 640 (4x cheaper than feature-major). bf16 stationary
    triggers Fast Weight Load (128 cols).
  - L3 matmuls are GONE: logit[b,t] = sum_h wf_h * a2[b,t,h] is a DVE
    multiply + segmented reduce straight into the [128 b, t] softmax layout
    (SBUF, no PSUM needed).
  - Emission is software-pipelined so ACT (the bottleneck engine) never stalls.
"""

import contextlib

import numpy as np

import concourse.bacc as bacc
import concourse.mybir as mybir
import concourse.tile as tile
from concourse.bass_utils import run_bass_kernel_spmd

N_CORES = 8
B, T, D = 4096, 200, 32
BC = B // N_CORES  # 512 rows per core
H1, H2 = 80, 40
NBLK = BC // 128   # 4 blocks of 128 b's
NEG_BIG = float(np.float32(-2.0**32 + 1.0))

S2_PADDED = True   # sigma2 reads the full padded [.,1024] span vs strided 640
S2_TIGHT = False   # 40-stride ps2 slots (bank-edge split): a wash vs padded on HW
TAIL_PIPE = True   # per-block softmax tail (DVE sums) vs monolithic ACT accum
K_BF16 = True      # carry k / qp / qk / W1 through the L1 matmuls in bf16

F32 = mybir.dt.float32
F32R = mybir.dt.float32r
BF16 = mybir.dt.bfloat16
I8 = mybir.dt.int8
AF = mybir.ActivationFunctionType
ALU = mybir.AluOpType
AX = mybir.AxisListType


def _emit(nc, tc, es, d, TT, repeat=1):
    NG = TT // 4
    const = es.enter_context(tc.tile_pool(name="const", bufs=1))
    ktp = es.enter_context(tc.tile_pool(name="ktp", bufs=4))
    qkp = es.enter_context(tc.tile_pool(name="qkp", bufs=3))
    a2p = es.enter_context(tc.tile_pool(name="a2p", bufs=2))
    prp = es.enter_context(tc.tile_pool(name="prp", bufs=2))
    ps1p = es.enter_context(tc.tile_pool(name="ps1p", bufs=1, space="PSUM"))
    ps2p = es.enter_context(tc.tile_pool(name="ps2p", bufs=1, space="PSUM"))

    # ---- static tiles ----
    w1raw = const.tile([4 * D, H1], F32R)
    wrep = const.tile([128, 3 * H1], F32R)  # strip j: [W1q+W1m | W1k-W1m | W1p]
    wq = const.tile([D, D], F32R)
    w2raw = const.tile([H1 + 1, H2], F32)
    w2aug = const.tile([H1 + 1, H2], BF16)  # [0.5*W2 ; c2] (bias via ones-row)
    wfraw = const.tile([128, H2], F32)
    wfb = const.tile([128, H2], BF16)       # 0.5*Wf replicated on all partitions
    b1h = const.tile([H1, 1], F32)          # b1 / 2
    bqs = const.tile([D, 1], F32)
    als = const.tile([D, 1], F32)
    qts = const.tile([D, BC], F32R)
    qp4 = const.tile([128, BC], F32R)       # qp^T replicated at 4 strips
    mki = const.tile([128, NBLK, TT], I8)
    negb = const.tile([128, NBLK, TT], F32)
    tmpr = const.tile([D, BC], F32)
    tmpa = const.tile([D, BC], F32)
    tmpb = const.tile([D, BC], F32)
    # a1 double buffer: [81, 4, BC] bf16, row 80 = ones (bias row)
    a1A = const.tile([H1 + 1, 4, BC], BF16)
    a1B = const.tile([H1 + 1, 4, BC], BF16)
    logt = const.tile([128, NBLK, TT], F32)
    mx = const.tile([128, NBLK], F32)
    sums = const.tile([128, NBLK], F32)
    rin = const.tile([128, NBLK], F32)
    expv = const.tile([128, NBLK, TT], F32)
    att = const.tile([128, NBLK, TT], F32)

    nc.sync.dma_start(out=w1raw, in_=d["W1"])
    nc.sync.dma_start(out=wq, in_=d["Wq"])
    nc.sync.dma_start(out=w2raw, in_=d["W2aug"])
    nc.sync.dma_start(out=wfraw, in_=d["wfb"])
    nc.vector.tensor_copy(w2aug, w2raw)
    nc.vector.tensor_copy(wfb, wfraw)
    nc.sync.dma_start(out=b1h, in_=d["b1h"])
    nc.sync.dma_start(out=bqs, in_=d["bq"])
    nc.sync.dma_start(out=als, in_=d["alpha"])
    nc.sync.dma_start(out=qts, in_=d["qT"])
    nc.sync.dma_start(out=mki, in_=d["mki"])
    # ones bias-row at partition 80: DVE can't start mid-strip -> memset at
    # partition 0 and DMA the row into place
    ones1 = const.tile([1, 4 * BC], BF16)
    nc.vector.memset(ones1, 1.0)
    nc.sync.dma_start(
        out=a1A[H1:H1 + 1, :, :], in_=ones1.rearrange("p (j b) -> p j b", j=4))
    nc.sync.dma_start(
        out=a1B[H1:H1 + 1, :, :], in_=ones1.rearrange("p (j b) -> p j b", j=4))

    # dummy tanh: hoists the activation-table load (exp_and_others covers both
    # Tanh and Exp) into setup so the first real tanh doesn't serialize on it
    nc.scalar.activation(tmpr[:, 0:1], bqs, AF.Tanh)

    # combined W1 blocks, then replicate to strips 1..3.
    # HW verifier (NCC_IBIR297) requires equal base partitions for 2-input DVE
    # ops, so align the blocks to base 0 first; cross-partition moves go via DMA.
    t32 = const.tile([32, H1], F32R)
    t64 = const.tile([32, H1], F32R)
    nc.sync.dma_start(out=t32, in_=d["W1"][32:64, :])
    nc.sync.dma_start(out=t64, in_=d["W1"][64:96, :])
    nc.vector.tensor_add(wrep[0:32, 0:H1], w1raw[0:32, :], t64)
    nc.vector.tensor_sub(wrep[0:32, H1:2 * H1], t32, t64)
    nc.sync.dma_start(out=wrep[0:32, 2 * H1:3 * H1], in_=d["W1"][96:128, :])
    for j in range(1, 4):
        nc.sync.dma_start(out=wrep[32 * j:32 * j + 32, :], in_=wrep[0:32, :])

    # qp^T = prelu(Wq^T @ q^T + bq, alpha)
    ps0 = ps1p.tile([D, BC], F32, tag="ps1")
    nc.tensor.matmul(ps0, wq, qts, start=True, stop=True)
    nc.vector.tensor_scalar(tmpr, ps0, bqs, 0.0, op0=ALU.add, op1=ALU.max)
    nc.vector.tensor_scalar(tmpa, ps0, bqs, 0.0, op0=ALU.add, op1=ALU.min)
    nc.vector.tensor_scalar(tmpb, tmpa, als, None, op0=ALU.mult)
    nc.vector.tensor_add(qp4[0:32, :], tmpr, tmpb)
    for j in range(1, 4):
        nc.sync.dma_start(out=qp4[32 * j:32 * j + 32, :], in_=qp4[0:32, :])

    nc.vector.memset(negb, NEG_BIG)

    if K_BF16:
        wrep_b = const.tile([128, 3 * H1], BF16)
        qp4_b = const.tile([128, BC], BF16)
        nc.vector.tensor_copy(wrep_b, wrep.bitcast(F32))
        nc.vector.tensor_copy(qp4_b, qp4.bitcast(F32))
    else:
        wrep_b, qp4_b = wrep, qp4

    # persistent PSUM tiles. Matmul outputs must not cross a PSUM bank boundary:
    # either pad slots to 64 ([128, j, c, 64]) or pack slots at 40 and split the
    # one bank-edge-crossing slot's matmul in two ([128, 16, 40], S2_TIGHT).
    ps1 = ps1p.tile([H1, 4, BC], F32, tag="ps1")       # 4 banks
    if S2_TIGHT:
        ps2A = ps2p.tile([128, 16, H2], F32)           # 640 f32 -> 2 banks
        ps2B = ps2p.tile([128, 16, H2], F32)
    else:
        ps2A = ps2p.tile([128, 4, NBLK, 64], F32)      # 2 banks
        ps2B = ps2p.tile([128, 4, NBLK, 64], F32)
    # (padded layout) pad columns are never matmul-written; zero once so
    # tanh(junk) can't NaN
    nc.vector.memset(ps2A, 0.0)
    nc.vector.memset(ps2B, 0.0)

    for _rep in range(repeat):
        _main_pass(nc, d, TT, NG, ktp, qkp, a2p, prp, ps1, (ps2A, ps2B),
                   (a1A, a1B), logt, wrep_b, w2aug, wfb, b1h, qp4_b, mki, negb,
                   mx, sums, rin, expv, att)


def _main_pass(nc, d, TT, NG, ktp, qkp, a2p, prp, ps1, ps2s, a1s_ab, logt,
               wrep, w2aug, wfb, b1h, qp4, mki, negb, mx, sums, rin, expv, att):
    kts, qks, a2s = {}, {}, {}

    KD = BF16 if K_BF16 else F32R

    def dma_kt(g):
        kt = ktp.tile([128, BC], KD)
        nc.sync.dma_start(
            out=kt, in_=d["kT"][4 * g:4 * g + 4].rearrange("tj f b -> (tj f) b"))
        kts[g] = kt

    def emit_qk(g):
        qk = qkp.tile([128, BC], KD)
        nc.vector.tensor_mul(qk, qp4, kts[g])
        qks[g] = qk

    def emit_l1(g):
        kt, qk = kts.pop(g), qks.pop(g)
        for j in range(4):
            s = slice(32 * j, 32 * j + 32)
            tp = (32 * j, 0)
            p1 = ps1[:, j, :]
            nc.tensor.matmul(p1, wrep[s, 0:H1], qp4[s, :], start=True,
                             stop=False, tile_position=tp)
            nc.tensor.matmul(p1, wrep[s, H1:2 * H1], kt[s, :], start=False,
                             stop=False, tile_position=tp)
            nc.tensor.matmul(p1, wrep[s, 2 * H1:3 * H1], qk[s, :], start=False,
                             stop=True, tile_position=tp)

    def emit_s1(g):
        a1 = a1s_ab[g % 2]
        nc.scalar.activation(a1[0:H1, :, :], ps1, AF.Tanh, bias=b1h, scale=0.5)

    def emit_l2(g):
        a1 = a1s_ab[g % 2]
        ps2 = ps2s[g % 2]
        for j in range(4):
            for c in range(NBLK):
                lhsT = a1[:, j, 128 * c:128 * c + 128]
                if not S2_TIGHT:
                    nc.tensor.matmul(ps2[:, j, c, 0:H2], lhsT, w2aug,
                                     start=True, stop=True)
                    continue
                s = 4 * j + c
                lo = s * H2
                cut = 512 - lo  # elements of this slot left in the lower bank
                if 0 < cut < H2:
                    nc.tensor.matmul(ps2[:, s, 0:cut], lhsT, w2aug[:, 0:cut],
                                     start=True, stop=True)
                    nc.tensor.matmul(ps2[:, s, cut:H2], lhsT, w2aug[:, cut:H2],
                                     start=True, stop=True)
                else:
                    nc.tensor.matmul(ps2[:, s, :], lhsT, w2aug,
                                     start=True, stop=True)

    def emit_s2(g):
        ps2 = ps2s[g % 2]
        if S2_TIGHT:
            a2 = a2p.tile([128, 16, H2], BF16)
            nc.scalar.activation(a2, ps2, AF.Tanh, scale=0.5)
        else:
            a2 = a2p.tile([128, 4, NBLK, 64], BF16)
            if S2_PADDED:
                nc.scalar.activation(a2, ps2, AF.Tanh, scale=0.5)
            else:
                nc.scalar.activation(a2[:, :, :, 0:H2], ps2[:, :, :, 0:H2],
                                     AF.Tanh, scale=0.5)
        a2s[g] = a2

    def emit_logits(g):
        a2 = a2s.pop(g)
        if S2_TIGHT:
            pr = prp.tile([128, 16, H2], BF16)
            nc.vector.tensor_mul(
                pr, a2, wfb.unsqueeze(1).broadcast_to([128, 16, H2]))
            prv = pr.rearrange("p (j c) h -> p j c h", j=4)
        else:
            pr = prp.tile([128, 4, NBLK, 64], BF16)
            nc.vector.tensor_mul(
                pr[:, :, :, 0:H2], a2[:, :, :, 0:H2],
                wfb.unsqueeze(1).unsqueeze(1).broadcast_to([128, 4, NBLK, H2]))
            prv = pr[:, :, :, 0:H2]
        # out view [128, j, c] of logt[:, c, t0+j] via free-dim transpose
        nc.vector.tensor_reduce(
            logt[:, :, 4 * g:4 * g + 4].transpose([0, 2, 1]),
            prv, axis=AX.X, op=ALU.add)

    # ---- software-pipelined main loop ----
    dma_kt(0)
    emit_qk(0)
    emit_l1(0)
    emit_s1(0)
    for g in range(NG):
        if g + 1 < NG:
            dma_kt(g + 1)
            emit_qk(g + 1)
            emit_l1(g + 1)
        emit_l2(g)
        if g >= 1:
            emit_logits(g - 1)
        if g + 1 < NG:
            emit_s1(g + 1)
        emit_s2(g)
    emit_logits(NG - 1)

    # ---- masked softmax over t ----
    # Per-block pipeline: exp (ACT) || sum+recip+scale (DVE) || out-DMA of the
    # previous block. No ACT accum_out (its read-accumulator aux op is ~0.7us);
    # the sum runs on the otherwise-idle DVE instead.
    outv = d["out"].rearrange("(blk p) t -> p blk t", blk=NBLK)
    nc.vector.copy_predicated(logt, mki, negb)
    nc.vector.tensor_reduce(mx, logt, axis=AX.X, op=ALU.max, negate=True)
    if TAIL_PIPE:
        for blk in range(NBLK):
            nc.scalar.activation(
                expv[:, blk, :],
                logt[:, blk, :],
                AF.Exp,
                bias=mx[:, blk:blk + 1],
            )
            nc.vector.tensor_reduce(
                sums[:, blk:blk + 1], expv[:, blk, :], axis=AX.X, op=ALU.add)
            nc.vector.reciprocal(rin[:, blk:blk + 1], sums[:, blk:blk + 1])
            nc.vector.tensor_scalar(
                att[:, blk, :], expv[:, blk, :], rin[:, blk:blk + 1], None,
                op0=ALU.mult)
            nc.sync.dma_start(out=outv[:, blk, :], in_=att[:, blk, :])
    else:
        for blk in range(NBLK):
            nc.scalar.activation(
                expv[:, blk, :], logt[:, blk, :], AF.Exp,
                bias=mx[:, blk:blk + 1], accum_out=sums[:, blk:blk + 1])
        nc.vector.reciprocal(rin, sums)
        for blk in range(NBLK):
            nc.vector.tensor_scalar(
                att[:, blk, :], expv[:, blk, :], rin[:, blk:blk + 1], None,
                op0=ALU.mult)
        nc.sync.dma_start(out=outv, in_=att)


def build(TT=T, repeat=1):
    nc = bacc.Bacc("TRN2", target_bir_lowering=False, debug=False,
                   num_devices=N_CORES)
    d = {
        "kT": nc.dram_tensor("kT", [TT, D, BC], BF16 if K_BF16 else F32R,
                             kind="ExternalInput").ap(),
        "qT": nc.dram_tensor("qT", [D, BC], F32R, kind="ExternalInput").ap(),
        "mki": nc.dram_tensor("mki", [128, NBLK, TT], I8, kind="ExternalInput").ap(),
        "Wq": nc.dram_tensor("Wq", [D, D], F32R, kind="ExternalInput").ap(),
        "bq": nc.dram_tensor("bq", [D, 1], F32, kind="ExternalInput").ap(),
        "alpha": nc.dram_tensor("alpha", [D, 1], F32, kind="ExternalInput").ap(),
        "W1": nc.dram_tensor("W1", [4 * D, H1], F32R, kind="ExternalInput").ap(),
        "b1h": nc.dram_tensor("b1h", [H1, 1], F32, kind="ExternalInput").ap(),
        "W2aug": nc.dram_tensor("W2aug", [H1 + 1, H2], F32,
                                kind="ExternalInput").ap(),
        "wfb": nc.dram_tensor("wfb", [128, H2], F32, kind="ExternalInput").ap(),
        "out": nc.dram_tensor("out", [BC, TT], F32, kind="ExternalOutput").ap(),
    }
    with tile.TileContext(nc) as tc:
        with contextlib.ExitStack() as es:
            _emit(nc, tc, es, d, TT, repeat=repeat)
    nc.compile()
    return nc


def prepare(q, k, mask, Wq, bq, alpha, W1, b1, W2, b2, Wf, bf=None):
    """Varlen packing: per batch row keep only its unmasked t's (plus padding to
    the global max count, rounded to a multiple of 4). Pure index manipulation.
    Returns (in_maps, TT, tidx)."""
    mask_np = np.asarray(mask)
    cnt = (mask_np != 0).sum(1)                      # unmasked count per row
    if cnt.min() == 0:
        # a fully-masked row needs the uniform-softmax semantics; identity
        # "compaction" reproduces the uncompacted kernel exactly
        TT = T
        tidx = np.ascontiguousarray(np.tile(np.arange(T), (B, 1)))
        pad = mask_np == 0
    else:
        TT = int(-(-int(cnt.max()) // 4) * 4)        # round up to x4
        TT = max(TT, 8)
        order = np.argsort(mask_np == 0, axis=1, kind="stable")  # unmasked first
        tidx = np.ascontiguousarray(order[:, :TT])   # [B, TT]
        pad = (np.arange(TT)[None, :] >= cnt[:, None])  # True on pad slots
    kc = np.take_along_axis(np.asarray(k), tidx[:, :, None], axis=1)  # [B, TT, D]

    W2 = np.asarray(W2, np.float32)
    b2 = np.asarray(b2, np.float32)
    Wf = np.asarray(Wf, np.float32).reshape(H2)
    b1 = np.asarray(b1, np.float32)
    # sigma(x) = 0.5 + 0.5*tanh(x/2) folding:
    #   z2 = W2^T sig1 + b2 = (0.5 W2)^T t1 + (b2 + 0.5 * sum_h W2[h,:])
    #   logit = Wf . sig2 (+const) = (0.5 Wf) . t2 + const   (const drops in softmax)
    c2 = b2 + 0.5 * W2.sum(axis=0)                   # [H2]
    W2aug = np.concatenate([0.5 * W2, c2[None, :]], axis=0)  # [81, 40]
    wfb = np.tile((0.5 * Wf)[None, :], (128, 1))     # [128, 40]

    common = {
        "Wq": np.ascontiguousarray(Wq, np.float32),
        "bq": np.ascontiguousarray(bq, np.float32).reshape(D, 1),
        "alpha": np.ascontiguousarray(alpha, np.float32).reshape(D, 1),
        "W1": np.ascontiguousarray(W1, np.float32),
        "b1h": np.ascontiguousarray(0.5 * b1, np.float32).reshape(H1, 1),
        "W2aug": np.ascontiguousarray(W2aug, np.float32),
        "wfb": np.ascontiguousarray(wfb, np.float32),
    }
    in_maps = []
    for c in range(N_CORES):
        sl = slice(c * BC, (c + 1) * BC)
        if K_BF16:
            import ml_dtypes
            kcc = np.ascontiguousarray(
                kc[sl].transpose(1, 2, 0).astype(ml_dtypes.bfloat16))
        else:
            kcc = np.ascontiguousarray(kc[sl].transpose(1, 2, 0), np.float32)
        qc = np.ascontiguousarray(np.asarray(q)[sl].T, np.float32)  # [D, BC]
        mc = pad[sl].astype(np.int8)
        mc = np.ascontiguousarray(mc.reshape(NBLK, 128, TT).transpose(1, 0, 2))
        m = dict(common)
        m.update({"kT": kcc, "qT": qc, "mki": mc})
        in_maps.append(m)
    return in_maps, TT, tidx


def postprocess(results, TT, tidx):
    attc = np.empty((B, TT), np.float32)
    for c in range(N_CORES):
        attc[c * BC:(c + 1) * BC] = results[c]["out"]
    out = np.zeros((B, T), np.float32)
    np.put_along_axis(out, tidx, attc, axis=1)
    return out.reshape(B, 1, T)


_NC_CACHE = {}


def kernel(**inputs):
    in_maps, TT, tidx = prepare(
        inputs["q"], inputs["k"], inputs["mask"], inputs["Wq"], inputs["bq"],
        inputs["alpha"], inputs["W1"], inputs["b1"], inputs["W2"], inputs["b2"],
        inputs["Wf"],
    )
    if TT not in _NC_CACHE:
        _NC_CACHE[TT] = build(TT=TT)
    nc = _NC_CACHE[TT]
    res = run_bass_kernel_spmd(nc, in_maps, core_ids=list(range(N_CORES)))
    return postprocess(res.results, TT, tidx)
